# revision 24
# baseline (speedup 1.0000x reference)
"""ComplexMultiHeadAttention on 8 TRN2 NeuronCores (Bass/Tile) — fused stream.

Problem: B=4, S=1024, D_MODEL=1024, N_HEADS=16, D_HEAD=64, complex-valued
activations stored as a trailing dim of size 2 (real, imag).

    q = to_heads(complex_linear(queries, wq));  k, v likewise
    s_r + i*s_i = (q_r + i q_i)(k_r + i k_i)^T / sqrt(dh)
    a_r = softmax(s_r), a_i = softmax(s_i)      (independent softmaxes)
    o = complex_bmm(a, v);  out = complex_linear(concat_heads(o), wo)

Sharding: head-parallel. Core c owns heads {2c, 2c+1} = 128 contiguous dims
of the hidden axis. Weights row-sliced for QKV, wo column-sliced; the host
sums the 8 partial outputs — no on-device collectives.

Key design points (TRN2):
  - ONE fused tensor stream: attention(b) is interleaved, per key-chunk
    unit, with "filler" matmuls from oproj(b-1) and the q/k/v projections
    of (b+1).  Every engine's work is spread over the whole batch window,
    so no phase boundary ever idles the PE (which would also drop the
    DVFS p-state to half rate for ~5us).
  - V is projected directly in TRANSPOSED form: V^T = X^T W per 128-token
    block (X slice stationary, weights moving, 256-wide streams).  The
    value matrix lands token-major straight out of the PE — no DMA
    transposes, nothing on the scalar queue but the exps.
  - vb elimination: ob = va^T u_i and the epilogue reads its halves
    swapped (the complex cross terms only differ by that swap).
  - Z (softmax denominators): u chunks are accumulated with bf16 vector
    adds into u_acc per (head, q-half); ONE ones-matmul pair per group
    replaces 8 — Z tensor cols drop 8x and two PSUM banks are freed,
    which is exactly what lets scores/AV/aux all fit in 8 banks.
  - PSUM: scores 2x2 banks, AV wide 1x2 banks, aux (proj/oproj/Z) 2x1.
  - All matmuls bf16 (f32 PSUM accumulation); softmax over keys skips
    max-subtraction (scores are O(1) by construction).
"""

import os
import numpy as np
import ml_dtypes
from contextlib import ExitStack

import concourse.bass as bass
import concourse.tile as tile
from concourse import bacc, mybir

F32 = mybir.dt.float32
BF16 = mybir.dt.bfloat16
EXP = mybir.ActivationFunctionType.Exp

B, S, D, H, DH = 4, 1024, 1024, 16, 64
NCORES = 8
P = 128            # partitions / chunk size
TBLK = 512         # token block (matmul free dim)
WBLK = 2 * TBLK    # wide tile (2 psum banks)
DC = D // P        # 8 d-chunks
KC = S // P        # 8 key chunks per batch
HPC = H // NCORES  # 2 heads per core
NT = (B * S) // TBLK
NUNITS = HPC * 2 * KC  # 32 attention units per batch

_CACHE = {}


def _build():
    nc = bacc.Bacc("TRN2", target_bir_lowering=False, debug=False,
                   num_devices=NCORES)

    # partition-major tiled layout: row gt*128+p, col dc*512+tok
    x_ap = {}
    for t in ("q", "k", "v"):
        for part in ("r", "i"):
            x_ap[t + part] = nc.dram_tensor(
                f"x{t}_{part}", [NT * P, DC * TBLK],
                BF16, kind="ExternalInput").ap()
    w_ap = {}
    for t in ("q", "k"):
        for h in range(HPC):
            for suf in ("a", "b"):
                w_ap[f"{t}{suf}{h}"] = nc.dram_tensor(
                    f"w{t}_{suf}{h}", [P, D], BF16, kind="ExternalInput").ap()
    wvt_ap = {}
    for part in ("r", "i"):
        wvt_ap[part] = nc.dram_tensor(
            f"wvt_{part}", [P, DC * 2 * P], BF16, kind="ExternalInput").ap()
    wo_ap = {}
    for suf in ("r", "i", "in"):
        wo_ap[suf] = nc.dram_tensor(
            f"wo_{suf}", [P, D], BF16, kind="ExternalInput").ap()
    ones_ap = nc.dram_tensor("onesin", [P, P], BF16, kind="ExternalInput").ap()
    # output: row gt*128+p, col (2*mc+ri)*512+tok  (r/i interleaved per mc)
    po_ap = nc.dram_tensor("po", [NT * P, 2 * DC * TBLK], BF16,
                           kind="ExternalOutput").ap()

    with tile.TileContext(nc) as tc, ExitStack() as ctx:
        wpool = ctx.enter_context(tc.tile_pool(name="w", bufs=1))
        xpool = ctx.enter_context(tc.tile_pool(name="x", bufs=8))
        qkpool = ctx.enter_context(tc.tile_pool(name="qk", bufs=2))
        vpool = ctx.enter_context(tc.tile_pool(name="v", bufs=2))
        opool = ctx.enter_context(tc.tile_pool(name="ost", bufs=2))
        upool = ctx.enter_context(tc.tile_pool(name="u", bufs=3))
        uaccpool = ctx.enter_context(tc.tile_pool(name="uacc", bufs=2))
        zpool = ctx.enter_context(tc.tile_pool(name="z", bufs=2))
        tmppool = ctx.enter_context(tc.tile_pool(name="tmp", bufs=2))
        popool = ctx.enter_context(tc.tile_pool(name="po", bufs=2))
        # PSUM: scores 2x2 banks + AV wide 1x2 banks + aux 2x1 bank = 8
        sps = ctx.enter_context(tc.tile_pool(name="sp", bufs=2, space="PSUM"))
        avps = ctx.enter_context(tc.tile_pool(name="av", bufs=1,
                                              space="PSUM"))
        auxps = ctx.enter_context(tc.tile_pool(name="ax", bufs=2,
                                               space="PSUM"))

        # Startup DMAs: weights on the scalar HWDGE queue in consumption
        # order (wvt, wq, wk, wo, ones), x loads on the sync queue in
        # consumption order — the two queues stream from HBM in parallel
        # and each arrival matches its consumer.
        wt = {}
        wot = {}
        wvt = {}
        xtiles = {}

        def load_w(dst, key, ap, pfx):
            dst[key] = wpool.tile(list(ap.shape), BF16, tag=f"{pfx}_{key}",
                                  name=f"{pfx}_{key}")
            nc.scalar.dma_start(dst[key][:], ap[:])

        def emit_xloads(b, t):
            for half in range(2):
                for part in ("r", "i"):
                    gt = 2 * b + half
                    xt = xpool.tile([P, DC * TBLK], BF16, tag="xt",
                                    name="xt")
                    nc.sync.dma_start(
                        xt[:], x_ap[t + part][gt * P:(gt + 1) * P, :])
                    xtiles[(b, t, part, half)] = xt

        for part in ("r", "i"):
            load_w(wvt, part, wvt_ap[part], "wvt")
        for h in range(HPC):
            for suf in ("a", "b"):
                load_w(wt, f"q{suf}{h}", w_ap[f"q{suf}{h}"], "w")
        for h in range(HPC):
            for suf in ("a", "b"):
                load_w(wt, f"k{suf}{h}", w_ap[f"k{suf}{h}"], "w")
        for suf, ap in wo_ap.items():
            load_w(wot, suf, ap, "wo")
        ones = wpool.tile([P, P], BF16, tag="ones", name="ones")
        nc.scalar.dma_start(ones[:], ones_ap[:])
        for t in ("v", "q", "k"):
            emit_xloads(0, t)

        def proj_gen(b, qcat, kcr, kci, va):
            """Projection of batch b as a stream of tensor quanta.

            Yields the emitted tensor-column count after each quantum.
            v first (V^T form), then q, then k; trailing vector ops of a
            psum group are emitted with its final quantum.
            """
            for half in range(2):
                xr = xtiles.pop((b, "v", "r", half))
                xi = xtiles.pop((b, "v", "i", half))
                for tb in range(4):
                    kc = half * 4 + tb
                    vps = auxps.tile([P, 2 * P], F32, tag="aux", name="vps")
                    for dcg in range(2):
                        for dc in range(dcg * 4, dcg * 4 + 4):
                            xs_ = slice(dc * TBLK + tb * P,
                                        dc * TBLK + (tb + 1) * P)
                            ws = slice(dc * 2 * P, (dc + 1) * 2 * P)
                            nc.tensor.matmul(
                                vps[:], xr[:, xs_], wvt["r"][:, ws],
                                start=(dc == 0), stop=False)
                            nc.tensor.matmul(
                                vps[:], xi[:, xs_], wvt["i"][:, ws],
                                start=False, stop=(dc == DC - 1))
                        if dcg == 1:
                            nc.vector.tensor_copy(
                                va[:, kc * 2 * P:(kc + 1) * 2 * P], vps[:])
                        yield 2048
            for t in ("q", "k"):
                for half in range(2):
                    xr = xtiles.pop((b, t, "r", half))
                    xi = xtiles.pop((b, t, "i", half))
                    hs = slice(half * TBLK, (half + 1) * TBLK)
                    for hh in range(2):
                        ps = auxps.tile([P, TBLK], F32, tag="aux",
                                        name="qkps")
                        wA = wt[f"{t}a{hh}"]
                        wB = wt[f"{t}b{hh}"]
                        for dcg in range(2):
                            for dc in range(dcg * 4, dcg * 4 + 4):
                                ws = slice(dc * P, (dc + 1) * P)
                                xs_ = slice(dc * TBLK, (dc + 1) * TBLK)
                                nc.tensor.matmul(
                                    ps[:], wA[:, ws], xr[:, xs_],
                                    start=(dc == 0), stop=False)
                                nc.tensor.matmul(
                                    ps[:], wB[:, ws], xi[:, xs_],
                                    start=False, stop=(dc == DC - 1))
                            if dcg == 1:
                                if t == "q":
                                    nc.vector.tensor_copy(qcat[hh][:, hs],
                                                          ps[:])
                                else:
                                    nc.vector.tensor_copy(kcr[hh][:, hs],
                                                          ps[:])
                                    nc.vector.tensor_scalar_mul(
                                        kci[hh][0:DH, hs], ps[DH:P, :], -1.0)
                                    nc.vector.tensor_copy(kci[hh][DH:P, hs],
                                                          ps[0:DH, :])
                            yield 4096

        def oproj_gen(b, o_stage, halves=(0, 1), pools=None):
            """O-projection of batch b as a stream of tensor quanta.

            powide copies run on the SCALAR engine (it can read PSUM and
            has slack) to keep the vector queue off the critical path.
            po is stored in quarter chunks (gpsimd SWDGE) so the final
            store's DMA tail is short and earlier chunks overlap compute.
            `pools`: psum pools to rotate over (drain mode passes the
            scores pool too, for 4 effective buffers).
            """
            if pools is None:
                pools = ((auxps, "aux"),)
            pi_ = 0
            for half in halves:
                gt = 2 * b + half
                hs = slice(half * TBLK, (half + 1) * TBLK)
                powide = popool.tile([P, 2 * DC * TBLK], BF16, tag="pow",
                                     name="powide")
                for mc in range(DC):
                    ms = slice(mc * P, (mc + 1) * P)
                    for ri in range(2):
                        pool, ptag = pools[pi_ % len(pools)]
                        pi_ += 1
                        ps = pool.tile([P, TBLK], F32, tag=ptag,
                                       name="ops")
                        if ri == 0:
                            pairs = ((wot["r"], o_stage["r"]),
                                     (wot["in"], o_stage["i"]))
                        else:
                            pairs = ((wot["i"], o_stage["r"]),
                                     (wot["r"], o_stage["i"]))
                        nc.tensor.matmul(ps[:], pairs[0][0][:, ms],
                                         pairs[0][1][:, hs],
                                         start=True, stop=False)
                        nc.tensor.matmul(ps[:], pairs[1][0][:, ms],
                                         pairs[1][1][:, hs],
                                         start=False, stop=True)
                        c0 = (2 * mc + ri) * TBLK
                        nc.scalar.copy(powide[:, c0:c0 + TBLK], ps[:])
                        yield 1024
                    if mc % 2 == 1:
                        cs = slice((2 * mc - 2) * TBLK, (2 * mc + 2) * TBLK)
                        nc.gpsimd.dma_start(po_ap[gt * P:(gt + 1) * P, cs],
                                            powide[:, cs])

        def drain(gen):
            for _ in gen:
                pass

        def emit_window(b, qcat, kcr, kci, va, o_stage, filler,
                        mid_filler=None):
            """Attention units of batch b with filler interleaved.

            qb-outer unit order: both heads' q-half epilogues complete by
            mid-window, so `mid_filler` (last batch's own half-0 oproj)
            can be injected after unit 15.
            """
            units = [(h, qb, kc)
                     for qb in range(2) for h in range(HPC)
                     for kc in range(KC)]
            total_fill = (32768 if b >= 1 else 0) + \
                         (98304 if b + 1 < B else 0)
            per_unit = (total_fill + NUNITS - 1) // NUNITS
            swides = [None] * len(units)
            accs = {}
            budget = 0

            def emit_scores(n):
                h, qb, kc = units[n]
                qs = slice(qb * TBLK, (qb + 1) * TBLK)
                ks = slice(kc * P, (kc + 1) * P)
                sw = sps.tile([P, WBLK], F32, tag="sps", name="scorew")
                nc.tensor.matmul(sw[:, 0:TBLK], kcr[h][:, ks],
                                 qcat[h][:, qs], start=True, stop=True)
                nc.tensor.matmul(sw[:, TBLK:WBLK], kci[h][:, ks],
                                 qcat[h][:, qs], start=True, stop=True)
                swides[n] = sw

            def emit_epilogue(h, qb, uacc, avw):
                # Z = ones^T u_acc (one matmul pair per (h,qb)), then
                # o_r = (v_r.T u_r)/Z_r - (v_i.T u_i)/Z_i etc.
                # ob (= avw cols TBLK:) is va^T u_i, halves swapped in
                # the combine; psum+sbuf DVE inputs are exempt from
                # the same-base-partition rule.
                qs = slice(qb * TBLK, (qb + 1) * TBLK)
                zps_r = auxps.tile([P, TBLK], F32, tag="aux", name="zpr")
                nc.tensor.matmul(zps_r[:], ones[:], uacc[:, 0:TBLK],
                                 start=True, stop=True)
                zps_i = auxps.tile([P, TBLK], F32, tag="aux", name="zpi")
                nc.tensor.matmul(zps_i[:], ones[:], uacc[:, TBLK:WBLK],
                                 start=True, stop=True)
                zinv = zpool.tile([P, WBLK], F32, tag="zinv", name="zinv")
                nc.vector.reciprocal_approx_fast(zinv[:, 0:TBLK], zps_r[:])
                nc.vector.reciprocal_approx_fast(zinv[:, TBLK:WBLK],
                                                 zps_i[:])
                tmpa = tmppool.tile([P, TBLK], F32, tag="tmpa", name="tmpa")
                nc.vector.tensor_mul(tmpa[:], avw[:, 0:TBLK],
                                     zinv[:, 0:TBLK])
                tmpb = tmppool.tile([P, TBLK], F32, tag="tmpb", name="tmpb")
                nc.vector.tensor_mul(tmpb[0:DH, :], avw[DH:P, TBLK:WBLK],
                                     zinv[DH:P, TBLK:WBLK])
                nc.vector.tensor_mul(tmpb[DH:P, :], avw[0:DH, TBLK:WBLK],
                                     zinv[0:DH, TBLK:WBLK])
                dst = slice(DH * h, DH * (h + 1))
                nc.vector.tensor_sub(o_stage["r"][dst, qs],
                                     tmpa[0:DH, :], tmpb[0:DH, :])
                nc.vector.tensor_add(o_stage["i"][dst, qs],
                                     tmpa[DH:P, :], tmpb[DH:P, :])

            pending = None
            emit_scores(0)
            for n, (h, qb, kc) in enumerate(units):
                if n + 1 < len(units):
                    emit_scores(n + 1)
                if n == NUNITS // 2 and mid_filler is not None:
                    filler.append(mid_filler)
                    per_unit += 1024
                first, last = kc == 0, kc == KC - 1
                if first:
                    uacc = uaccpool.tile([P, WBLK], BF16, tag="uacc",
                                         name="uacc")
                    avw = avps.tile([P, WBLK], F32, tag="av", name="avw")
                    accs[(h, qb)] = (uacc, avw)
                    u = uacc
                    nc.scalar.activation(uacc[:], swides[n][:], EXP)
                else:
                    uacc, avw = accs[(h, qb)]
                    u = upool.tile([P, WBLK], BF16, tag="u", name="u")
                    nc.scalar.activation(u[:], swides[n][:], EXP)
                    # u accumulation for Z on the (otherwise idle) gpsimd
                    # engine — SBUF-only op, keeps the vector queue short
                    nc.gpsimd.tensor_add(uacc[:], uacc[:], u[:])
                swides[n] = None
                # filler; kc==0 units get a bonus pop so the avw-reuse
                # boundary (previous group's deferred epilogue) is hidden
                budget += per_unit + (2048 if first else 0)
                while budget > 0 and filler:
                    try:
                        budget -= next(filler[0])
                    except StopIteration:
                        filler.pop(0)
                # previous group's Z + epilogue, deferred here (one unit of
                # extra slack for its exp -> u_acc add chain) but before
                # this AV so the single avw psum buffer frees in time
                if pending is not None:
                    emit_epilogue(*pending)
                    pending = None
                # AV
                vsl = va[:, kc * 2 * P + h * P:kc * 2 * P + (h + 1) * P]
                nc.tensor.matmul(avw[:, 0:TBLK], vsl, u[:, 0:TBLK],
                                 start=first, stop=last)
                nc.tensor.matmul(avw[:, TBLK:WBLK], vsl, u[:, TBLK:WBLK],
                                 start=first, stop=last)
                if last:
                    pending = (h, qb, uacc, avw)
                    del accs[(h, qb)]
            emit_epilogue(*pending)
            # drain leftover filler
            for g in filler:
                drain(g)

        # ---- pipelined emission: one continuous tensor stream ----
        stage = {}

        def new_stage(b):
            qcat = [qkpool.tile([P, S], BF16, tag=f"qcat{h}", name=f"qcat{h}")
                    for h in range(HPC)]
            kcr = [qkpool.tile([P, S], BF16, tag=f"kcr{h}", name=f"kcr{h}")
                   for h in range(HPC)]
            kci = [qkpool.tile([P, S], BF16, tag=f"kci{h}", name=f"kci{h}")
                   for h in range(HPC)]
            # va: [128 tokens-in-chunk, kc*256 + h*128 + [v_r(64)|v_i(64)]]
            va = vpool.tile([P, 2 * S], BF16, tag="va", name="va")
            o_stage = {p: opool.tile([P, S], BF16, tag=f"ost{p}",
                                     name=f"ost{p}")
                       for p in ("r", "i")}
            stage[b] = (qcat, kcr, kci, va, o_stage)

        new_stage(0)
        drain(proj_gen(0, *stage[0][:4]))
        for b in range(B):
            if b + 1 < B:
                for t in ("v", "q", "k"):
                    emit_xloads(b + 1, t)
                new_stage(b + 1)
            filler = []
            if b >= 1:
                filler.append(oproj_gen(b - 1, stage[b - 1][4]))
            if b + 1 < B:
                filler.append(proj_gen(b + 1, *stage[b + 1][:4]))
            # last window: inject this batch's own half-0 oproj once both
            # q-half-0 epilogues are in (qb-outer order, after unit 15)
            mid = (oproj_gen(b, stage[b][4], halves=(0,))
                   if b == B - 1 else None)
            emit_window(b, *stage[b], filler, mid_filler=mid)
            if b >= 1:
                del stage[b - 1]
        drain(oproj_gen(B - 1, stage[B - 1][4], halves=(1,),
                        pools=((auxps, "aux"), (sps, "sps"))))

    nc.compile()
    return nc


def _w_sbuf_layout(w_t):
    """[D, 128] weight-transpose slice -> SBUF layout [128, dc*128+o]."""
    return np.ascontiguousarray(
        w_t.reshape(DC, P, P).transpose(1, 0, 2).reshape(P, D))


def _tile_x(xT, dtype):
    """[D, B*S] -> partition-major [NT*P, DC*TBLK] (row gt*P+p, col dc*TBLK+t)."""
    t = xT.reshape(DC, P, NT, TBLK).transpose(2, 1, 0, 3)
    return np.ascontiguousarray(t.reshape(NT * P, DC * TBLK)).astype(dtype)


def _prepare_in_maps(inputs):
    bf = ml_dtypes.bfloat16
    xs = {}
    for name, t in (("queries", "q"), ("keys", "k"), ("values", "v")):
        x = np.asarray(inputs[name], dtype=np.float32)  # [B,S,D,2]
        flat = x.reshape(B * S, D, 2)
        xs[t + "r"] = _tile_x(flat[:, :, 0].T, bf)
        xs[t + "i"] = _tile_x(flat[:, :, 1].T, bf)

    scale = np.float32(1.0 / np.sqrt(DH))
    in_maps = []
    for c in range(NCORES):
        rows = slice(P * c, P * (c + 1))
        m = {}
        for t in ("q", "k", "v"):
            for part in ("r", "i"):
                m[f"x{t}_{part}"] = xs[t + part]
        for t, wr_name, wi_name in (("q", "wq_r", "wq_i"),
                                    ("k", "wk_r", "wk_i")):
            s = scale if t == "q" else np.float32(1.0)
            wr = np.asarray(inputs[wr_name], dtype=np.float32)[rows] * s
            wi = np.asarray(inputs[wi_name], dtype=np.float32)[rows] * s
            for h in range(HPC):
                hr = slice(DH * h, DH * (h + 1))
                if t == "q":
                    wa = np.concatenate([wr[hr].T, wi[hr].T], axis=1)
                    wb = np.concatenate([-wi[hr].T, wr[hr].T], axis=1)
                else:
                    wa = np.concatenate([wr[hr].T, -wi[hr].T], axis=1)
                    wb = np.concatenate([-wi[hr].T, -wr[hr].T], axis=1)
                m[f"w{t}_a{h}"] = _w_sbuf_layout(wa).astype(bf)
                m[f"w{t}_b{h}"] = _w_sbuf_layout(wb).astype(bf)
        # V^T weights, moving operand: [1024 d, 2 heads * (v_r 64 | v_i 64)]
        # chunked to [128, dc*256 + c]
        wvr = np.asarray(inputs["wv_r"], dtype=np.float32)[rows]  # [128,1024]
        wvi = np.asarray(inputs["wv_i"], dtype=np.float32)[rows]
        br = np.concatenate(
            [np.concatenate([wvr[DH * h:DH * (h + 1)].T,
                             wvi[DH * h:DH * (h + 1)].T], axis=1)
             for h in range(HPC)], axis=1)  # [1024, 256]
        bi = np.concatenate(
            [np.concatenate([-wvi[DH * h:DH * (h + 1)].T,
                             wvr[DH * h:DH * (h + 1)].T], axis=1)
             for h in range(HPC)], axis=1)
        m["wvt_r"] = np.ascontiguousarray(
            br.reshape(DC, P, 2 * P).transpose(1, 0, 2).reshape(
                P, DC * 2 * P)).astype(bf)
        m["wvt_i"] = np.ascontiguousarray(
            bi.reshape(DC, P, 2 * P).transpose(1, 0, 2).reshape(
                P, DC * 2 * P)).astype(bf)
        wo_r = np.asarray(inputs["wo_r"], dtype=np.float32)[:, rows]  # [D,128]
        wo_i = np.asarray(inputs["wo_i"], dtype=np.float32)[:, rows]
        m["wo_r"] = np.ascontiguousarray(wo_r.T).astype(bf)  # [128 d, 1024 m]
        m["wo_i"] = np.ascontiguousarray(wo_i.T).astype(bf)
        m["wo_in"] = np.ascontiguousarray(-wo_i.T).astype(bf)
        m["onesin"] = np.ones((P, P), dtype=bf)
        in_maps.append(m)
    return in_maps


LAST_RESULT = None


def _run(inputs, trace=False):
    global LAST_RESULT
    from concourse.bass_utils import run_bass_kernel_spmd
    if "nc" not in _CACHE:
        _CACHE["nc"] = _build()
    nc = _CACHE["nc"]
    in_maps = _prepare_in_maps(inputs)
    if trace:
        os.environ.pop("BASS_NEVER_TRACE", None)
    else:
        os.environ["BASS_NEVER_TRACE"] = "1"
    res = run_bass_kernel_spmd(nc, in_maps, core_ids=list(range(NCORES)),
                               trace=trace)
    LAST_RESULT = res
    # po rows gt*P+p, cols (2*mc+ri)*TBLK+tok
    acc = np.zeros((NT * P, 2 * DC * TBLK), np.float32)
    for c in range(NCORES):
        acc += res.results[c]["po"].astype(np.float32)

    t = acc.reshape(NT, P, DC, 2, TBLK)
    out = np.empty((B, S, D, 2), np.float32)
    for ri in range(2):
        # value at [gt, p, mc, ri, tok] = out_part[d=mc*128+p, gt*512+tok]
        comp = t[:, :, :, ri, :].transpose(2, 1, 0, 3).reshape(D, B * S)
        out[..., ri] = comp.T.reshape(B, S, D)
    return out


def kernel(**inputs):
    return _run(inputs, trace=False)


# revision 29
# speedup vs baseline: 1.1070x; 1.1070x over previous
"""ComplexMultiHeadAttention on 8 TRN2 NeuronCores (Bass/Tile) — fused stream.

Problem: B=4, S=1024, D_MODEL=1024, N_HEADS=16, D_HEAD=64, complex-valued
activations stored as a trailing dim of size 2 (real, imag).

    q = to_heads(complex_linear(queries, wq));  k, v likewise
    s_r + i*s_i = (q_r + i q_i)(k_r + i k_i)^T / sqrt(dh)
    a_r = softmax(s_r), a_i = softmax(s_i)      (independent softmaxes)
    o = complex_bmm(a, v);  out = complex_linear(concat_heads(o), wo)

Sharding: head-parallel. Core c owns heads {2c, 2c+1} = 128 contiguous dims
of the hidden axis. Weights row-sliced for QKV, wo column-sliced; the host
sums the 8 partial outputs — no on-device collectives.

Key design points (TRN2):
  - ONE fused tensor stream: attention(b) is interleaved, per key-chunk
    unit, with "filler" matmuls from oproj(b-1) and the q/k/v projections
    of (b+1).  Every engine's work is spread over the whole batch window,
    so no phase boundary ever idles the PE (which would also drop the
    DVFS p-state to half rate for ~5us).
  - V is projected directly in TRANSPOSED form: V^T = X^T W per 128-token
    block (X slice stationary, weights moving, 256-wide streams).  The
    value matrix lands token-major straight out of the PE — no DMA
    transposes, nothing on the scalar queue but the exps.
  - vb elimination: ob = va^T u_i and the epilogue reads its halves
    swapped (the complex cross terms only differ by that swap).
  - Z (softmax denominators): u chunks are accumulated with bf16 vector
    adds into u_acc per (head, q-half); ONE ones-matmul pair per group
    replaces 8 — Z tensor cols drop 8x and two PSUM banks are freed,
    which is exactly what lets scores/AV/aux all fit in 8 banks.
  - PSUM: scores 2x2 banks, AV wide 1x2 banks, aux (proj/oproj/Z) 2x1.
  - All matmuls bf16 (f32 PSUM accumulation); softmax over keys skips
    max-subtraction (scores are O(1) by construction).
"""

import os
import numpy as np
import ml_dtypes
from contextlib import ExitStack

import concourse.bass as bass
import concourse.tile as tile
from concourse import bacc, mybir

F32 = mybir.dt.float32
BF16 = mybir.dt.bfloat16
EXP = mybir.ActivationFunctionType.Exp

B, S, D, H, DH = 4, 1024, 1024, 16, 64
NCORES = 8
P = 128            # partitions / chunk size
TBLK = 512         # token block (matmul free dim)
WBLK = 2 * TBLK    # wide tile (2 psum banks)
DC = D // P        # 8 d-chunks
KC = S // P        # 8 key chunks per batch
HPC = H // NCORES  # 2 heads per core
NT = (B * S) // TBLK
NUNITS = HPC * 2 * KC  # 32 attention units per batch

_CACHE = {}


def _build():
    nc = bacc.Bacc("TRN2", target_bir_lowering=False, debug=False,
                   num_devices=NCORES)

    # partition-major tiled layout: row gt*128+p, col dc*512+tok
    x_ap = {}
    for t in ("q", "k", "v"):
        for part in ("r", "i"):
            x_ap[t + part] = nc.dram_tensor(
                f"x{t}_{part}", [NT * P, DC * TBLK],
                BF16, kind="ExternalInput").ap()
    w_ap = {}
    for t in ("q", "k"):
        for h in range(HPC):
            for suf in ("a", "b"):
                w_ap[f"{t}{suf}{h}"] = nc.dram_tensor(
                    f"w{t}_{suf}{h}", [P, D], BF16, kind="ExternalInput").ap()
    wvt_ap = {}
    for part in ("r", "i"):
        wvt_ap[part] = nc.dram_tensor(
            f"wvt_{part}", [P, DC * 2 * P], BF16, kind="ExternalInput").ap()
    wo_ap = {}
    for suf in ("r", "i", "in"):
        wo_ap[suf] = nc.dram_tensor(
            f"wo_{suf}", [P, D], BF16, kind="ExternalInput").ap()
    ones_ap = nc.dram_tensor("onesin", [P, P], BF16, kind="ExternalInput").ap()
    # output: row gt*128+p, col (2*mc+ri)*512+tok  (r/i interleaved per mc)
    po_ap = nc.dram_tensor("po", [NT * P, 2 * DC * TBLK], BF16,
                           kind="ExternalOutput").ap()

    with tile.TileContext(nc) as tc, ExitStack() as ctx:
        wpool = ctx.enter_context(tc.tile_pool(name="w", bufs=1))
        xpool = ctx.enter_context(tc.tile_pool(name="x", bufs=8))
        qkpool = ctx.enter_context(tc.tile_pool(name="qk", bufs=2))
        vpool = ctx.enter_context(tc.tile_pool(name="v", bufs=2))
        opool = ctx.enter_context(tc.tile_pool(name="ost", bufs=2))
        upool = ctx.enter_context(tc.tile_pool(name="u", bufs=3))
        uaccpool = ctx.enter_context(tc.tile_pool(name="uacc", bufs=2))
        zpool = ctx.enter_context(tc.tile_pool(name="z", bufs=2))
        tmppool = ctx.enter_context(tc.tile_pool(name="tmp", bufs=2))
        popool = ctx.enter_context(tc.tile_pool(name="po", bufs=2))
        # PSUM: scores 2x2 banks + AV wide 1x2 banks + aux 2x1 bank = 8
        sps = ctx.enter_context(tc.tile_pool(name="sp", bufs=2, space="PSUM"))
        avps = ctx.enter_context(tc.tile_pool(name="av", bufs=1,
                                              space="PSUM"))
        auxps = ctx.enter_context(tc.tile_pool(name="ax", bufs=2,
                                               space="PSUM"))

        # Startup DMAs: weights on the scalar HWDGE queue in consumption
        # order (wvt, wq, wk, wo, ones), x loads on the sync queue in
        # consumption order — the two queues stream from HBM in parallel
        # and each arrival matches its consumer.
        wt = {}
        wot = {}
        wvt = {}
        xtiles = {}

        def load_w(dst, key, ap, pfx):
            dst[key] = wpool.tile(list(ap.shape), BF16, tag=f"{pfx}_{key}",
                                  name=f"{pfx}_{key}")
            nc.scalar.dma_start(dst[key][:], ap[:])

        def emit_xloads(b, t):
            # r on the sync queue, i on the gpsimd queue: a single DMA
            # queue tops out near ~190 GB/s, two stream in parallel
            for half in range(2):
                for part, q in (("r", nc.sync), ("i", nc.gpsimd)):
                    gt = 2 * b + half
                    xt = xpool.tile([P, DC * TBLK], BF16, tag="xt",
                                    name="xt")
                    q.dma_start(
                        xt[:], x_ap[t + part][gt * P:(gt + 1) * P, :])
                    xtiles[(b, t, part, half)] = xt

        for part in ("r", "i"):
            load_w(wvt, part, wvt_ap[part], "wvt")
        for h in range(HPC):
            for suf in ("a", "b"):
                load_w(wt, f"q{suf}{h}", w_ap[f"q{suf}{h}"], "w")
        for h in range(HPC):
            for suf in ("a", "b"):
                load_w(wt, f"k{suf}{h}", w_ap[f"k{suf}{h}"], "w")
        for suf, ap in wo_ap.items():
            load_w(wot, suf, ap, "wo")
        ones = wpool.tile([P, P], BF16, tag="ones", name="ones")
        nc.scalar.dma_start(ones[:], ones_ap[:])
        for t in ("v", "q", "k"):
            emit_xloads(0, t)

        def proj_gen(b, qcat, kcr, kci, va):
            """Projection of batch b as a stream of tensor quanta.

            Yields the emitted tensor-column count after each quantum.
            v first (V^T form), then q, then k; trailing vector ops of a
            psum group are emitted with its final quantum.
            """
            for half in range(2):
                xr = xtiles.pop((b, "v", "r", half))
                xi = xtiles.pop((b, "v", "i", half))
                for tb in range(4):
                    kc = half * 4 + tb
                    vps = auxps.tile([P, 2 * P], F32, tag="aux", name="vps")
                    for dcg in range(2):
                        for dc in range(dcg * 4, dcg * 4 + 4):
                            xs_ = slice(dc * TBLK + tb * P,
                                        dc * TBLK + (tb + 1) * P)
                            ws = slice(dc * 2 * P, (dc + 1) * 2 * P)
                            nc.tensor.matmul(
                                vps[:], xr[:, xs_], wvt["r"][:, ws],
                                start=(dc == 0), stop=False)
                            nc.tensor.matmul(
                                vps[:], xi[:, xs_], wvt["i"][:, ws],
                                start=False, stop=(dc == DC - 1))
                        if dcg == 1:
                            nc.vector.tensor_copy(
                                va[:, kc * 2 * P:(kc + 1) * 2 * P], vps[:])
                        yield 2048
            for t in ("q", "k"):
                for half in range(2):
                    xr = xtiles.pop((b, t, "r", half))
                    xi = xtiles.pop((b, t, "i", half))
                    hs = slice(half * TBLK, (half + 1) * TBLK)
                    for hh in range(2):
                        ps = auxps.tile([P, TBLK], F32, tag="aux",
                                        name="qkps")
                        wA = wt[f"{t}a{hh}"]
                        wB = wt[f"{t}b{hh}"]
                        for dcg in range(2):
                            for dc in range(dcg * 4, dcg * 4 + 4):
                                ws = slice(dc * P, (dc + 1) * P)
                                xs_ = slice(dc * TBLK, (dc + 1) * TBLK)
                                nc.tensor.matmul(
                                    ps[:], wA[:, ws], xr[:, xs_],
                                    start=(dc == 0), stop=False)
                                nc.tensor.matmul(
                                    ps[:], wB[:, ws], xi[:, xs_],
                                    start=False, stop=(dc == DC - 1))
                            if dcg == 1:
                                if t == "q":
                                    nc.vector.tensor_copy(qcat[hh][:, hs],
                                                          ps[:])
                                else:
                                    nc.vector.tensor_copy(kcr[hh][:, hs],
                                                          ps[:])
                                    nc.vector.tensor_scalar_mul(
                                        kci[hh][0:DH, hs], ps[DH:P, :], -1.0)
                                    nc.vector.tensor_copy(kci[hh][DH:P, hs],
                                                          ps[0:DH, :])
                            yield 4096

        def oproj_gen(b, o_stage, halves=(0, 1), pools=None, alt_copy=False):
            """O-projection of batch b as a stream of tensor quanta.

            po is stored in quarter chunks (gpsimd SWDGE) so the final
            store's DMA tail is short and earlier chunks overlap compute.
            `pools`: psum pools to rotate over and `alt_copy` alternates
            powide copies between vector and scalar (drain mode: nothing
            else runs, so doubling both resources halves the drain).
            """
            if pools is None:
                pools = ((auxps, "aux"),)
            pi_ = 0
            for half in halves:
                gt = 2 * b + half
                hs = slice(half * TBLK, (half + 1) * TBLK)
                powide = popool.tile([P, 2 * DC * TBLK], BF16, tag="pow",
                                     name="powide")
                for mc in range(DC):
                    ms = slice(mc * P, (mc + 1) * P)
                    for ri in range(2):
                        pool, ptag = pools[pi_ % len(pools)]
                        pi_ += 1
                        ps = pool.tile([P, TBLK], F32, tag=ptag,
                                       name="ops")
                        if ri == 0:
                            pairs = ((wot["r"], o_stage["r"]),
                                     (wot["in"], o_stage["i"]))
                        else:
                            pairs = ((wot["i"], o_stage["r"]),
                                     (wot["r"], o_stage["i"]))
                        nc.tensor.matmul(ps[:], pairs[0][0][:, ms],
                                         pairs[0][1][:, hs],
                                         start=True, stop=False)
                        nc.tensor.matmul(ps[:], pairs[1][0][:, ms],
                                         pairs[1][1][:, hs],
                                         start=False, stop=True)
                        c0 = (2 * mc + ri) * TBLK
                        if alt_copy and ri == 1:
                            nc.scalar.copy(powide[:, c0:c0 + TBLK], ps[:])
                        else:
                            nc.vector.tensor_copy(powide[:, c0:c0 + TBLK],
                                                  ps[:])
                        yield 1024
                    if mc % 2 == 1:
                        cs = slice((2 * mc - 2) * TBLK, (2 * mc + 2) * TBLK)
                        nc.gpsimd.dma_start(po_ap[gt * P:(gt + 1) * P, cs],
                                            powide[:, cs])

        def drain(gen):
            for _ in gen:
                pass

        def emit_window(b, qcat, kcr, kci, va, o_stage, filler,
                        mid_filler=None):
            """Attention units of batch b with filler interleaved.

            qb-outer unit order: both heads' q-half epilogues complete by
            mid-window, so `mid_filler` (last batch's own half-0 oproj)
            can be injected after unit 15.
            """
            units = [(h, qb, kc)
                     for qb in range(2) for h in range(HPC)
                     for kc in range(KC)]
            total_fill = (32768 if b >= 1 else 0) + \
                         (98304 if b + 1 < B else 0)
            per_unit = (total_fill + NUNITS - 1) // NUNITS
            swides = [None] * len(units)
            accs = {}
            budget = 0

            def emit_scores(n):
                h, qb, kc = units[n]
                qs = slice(qb * TBLK, (qb + 1) * TBLK)
                ks = slice(kc * P, (kc + 1) * P)
                sw = sps.tile([P, WBLK], F32, tag="sps", name="scorew")
                nc.tensor.matmul(sw[:, 0:TBLK], kcr[h][:, ks],
                                 qcat[h][:, qs], start=True, stop=True)
                nc.tensor.matmul(sw[:, TBLK:WBLK], kci[h][:, ks],
                                 qcat[h][:, qs], start=True, stop=True)
                swides[n] = sw

            def emit_epilogue(h, qb, uacc, avw):
                # Z = ones^T u_acc (one matmul pair per (h,qb)), then
                # o_r = (v_r.T u_r)/Z_r - (v_i.T u_i)/Z_i etc.
                # ob (= avw cols TBLK:) is va^T u_i, halves swapped in
                # the combine; psum+sbuf DVE inputs are exempt from
                # the same-base-partition rule.
                qs = slice(qb * TBLK, (qb + 1) * TBLK)
                zps_r = auxps.tile([P, TBLK], F32, tag="aux", name="zpr")
                nc.tensor.matmul(zps_r[:], ones[:], uacc[:, 0:TBLK],
                                 start=True, stop=True)
                zps_i = auxps.tile([P, TBLK], F32, tag="aux", name="zpi")
                nc.tensor.matmul(zps_i[:], ones[:], uacc[:, TBLK:WBLK],
                                 start=True, stop=True)
                zinv = zpool.tile([P, WBLK], F32, tag="zinv", name="zinv")
                nc.vector.reciprocal_approx_fast(zinv[:, 0:TBLK], zps_r[:])
                nc.vector.reciprocal_approx_fast(zinv[:, TBLK:WBLK],
                                                 zps_i[:])
                tmpa = tmppool.tile([P, TBLK], F32, tag="tmpa", name="tmpa")
                nc.vector.tensor_mul(tmpa[:], avw[:, 0:TBLK],
                                     zinv[:, 0:TBLK])
                tmpb = tmppool.tile([P, TBLK], F32, tag="tmpb", name="tmpb")
                nc.vector.tensor_mul(tmpb[0:DH, :], avw[DH:P, TBLK:WBLK],
                                     zinv[DH:P, TBLK:WBLK])
                nc.vector.tensor_mul(tmpb[DH:P, :], avw[0:DH, TBLK:WBLK],
                                     zinv[0:DH, TBLK:WBLK])
                dst = slice(DH * h, DH * (h + 1))
                nc.vector.tensor_sub(o_stage["r"][dst, qs],
                                     tmpa[0:DH, :], tmpb[0:DH, :])
                nc.vector.tensor_add(o_stage["i"][dst, qs],
                                     tmpa[DH:P, :], tmpb[DH:P, :])

            pending = None
            emit_scores(0)
            for n, (h, qb, kc) in enumerate(units):
                if n + 1 < len(units):
                    emit_scores(n + 1)
                if n == NUNITS // 2 and mid_filler is not None:
                    filler.append(mid_filler)
                    per_unit += 1024
                first, last = kc == 0, kc == KC - 1
                if first:
                    uacc = uaccpool.tile([P, WBLK], BF16, tag="uacc",
                                         name="uacc")
                    avw = avps.tile([P, WBLK], F32, tag="av", name="avw")
                    accs[(h, qb)] = (uacc, avw)
                    u = uacc
                    nc.scalar.activation(uacc[:], swides[n][:], EXP)
                else:
                    uacc, avw = accs[(h, qb)]
                    u = upool.tile([P, WBLK], BF16, tag="u", name="u")
                    nc.scalar.activation(u[:], swides[n][:], EXP)
                    nc.vector.tensor_add(uacc[:], uacc[:], u[:])
                swides[n] = None
                # filler; kc==0 units get a bonus pop so the avw-reuse
                # boundary (previous group's deferred epilogue) is hidden
                budget += per_unit + (2048 if first else 0)
                while budget > 0 and filler:
                    try:
                        budget -= next(filler[0])
                    except StopIteration:
                        filler.pop(0)
                # previous group's Z + epilogue, deferred here (one unit of
                # extra slack for its exp -> u_acc add chain) but before
                # this AV so the single avw psum buffer frees in time
                if pending is not None:
                    emit_epilogue(*pending)
                    pending = None
                # AV
                vsl = va[:, kc * 2 * P + h * P:kc * 2 * P + (h + 1) * P]
                nc.tensor.matmul(avw[:, 0:TBLK], vsl, u[:, 0:TBLK],
                                 start=first, stop=last)
                nc.tensor.matmul(avw[:, TBLK:WBLK], vsl, u[:, TBLK:WBLK],
                                 start=first, stop=last)
                if last:
                    pending = (h, qb, uacc, avw)
                    del accs[(h, qb)]
            emit_epilogue(*pending)
            # drain leftover filler
            for g in filler:
                drain(g)

        # ---- pipelined emission: one continuous tensor stream ----
        stage = {}

        def new_stage(b):
            qcat = [qkpool.tile([P, S], BF16, tag=f"qcat{h}", name=f"qcat{h}")
                    for h in range(HPC)]
            kcr = [qkpool.tile([P, S], BF16, tag=f"kcr{h}", name=f"kcr{h}")
                   for h in range(HPC)]
            kci = [qkpool.tile([P, S], BF16, tag=f"kci{h}", name=f"kci{h}")
                   for h in range(HPC)]
            # va: [128 tokens-in-chunk, kc*256 + h*128 + [v_r(64)|v_i(64)]]
            va = vpool.tile([P, 2 * S], BF16, tag="va", name="va")
            o_stage = {p: opool.tile([P, S], BF16, tag=f"ost{p}",
                                     name=f"ost{p}")
                       for p in ("r", "i")}
            stage[b] = (qcat, kcr, kci, va, o_stage)

        new_stage(0)
        drain(proj_gen(0, *stage[0][:4]))
        for b in range(B):
            if b + 1 < B:
                for t in ("v", "q", "k"):
                    emit_xloads(b + 1, t)
                new_stage(b + 1)
            filler = []
            if b >= 1:
                filler.append(oproj_gen(b - 1, stage[b - 1][4]))
            if b + 1 < B:
                filler.append(proj_gen(b + 1, *stage[b + 1][:4]))
            # last window: inject this batch's own half-0 oproj once both
            # q-half-0 epilogues are in (qb-outer order, after unit 15)
            mid = (oproj_gen(b, stage[b][4], halves=(0,))
                   if b == B - 1 else None)
            emit_window(b, *stage[b], filler, mid_filler=mid)
            if b >= 1:
                del stage[b - 1]
        drain(oproj_gen(B - 1, stage[B - 1][4], halves=(1,),
                        pools=((auxps, "aux"), (sps, "sps")),
                        alt_copy=True))

    nc.compile()
    return nc


def _w_sbuf_layout(w_t):
    """[D, 128] weight-transpose slice -> SBUF layout [128, dc*128+o]."""
    return np.ascontiguousarray(
        w_t.reshape(DC, P, P).transpose(1, 0, 2).reshape(P, D))


def _tile_x(xT, dtype):
    """[D, B*S] -> partition-major [NT*P, DC*TBLK] (row gt*P+p, col dc*TBLK+t)."""
    t = xT.reshape(DC, P, NT, TBLK).transpose(2, 1, 0, 3)
    return np.ascontiguousarray(t.reshape(NT * P, DC * TBLK)).astype(dtype)


def _prepare_in_maps(inputs):
    bf = ml_dtypes.bfloat16
    xs = {}
    for name, t in (("queries", "q"), ("keys", "k"), ("values", "v")):
        x = np.asarray(inputs[name], dtype=np.float32)  # [B,S,D,2]
        flat = x.reshape(B * S, D, 2)
        xs[t + "r"] = _tile_x(flat[:, :, 0].T, bf)
        xs[t + "i"] = _tile_x(flat[:, :, 1].T, bf)

    scale = np.float32(1.0 / np.sqrt(DH))
    in_maps = []
    for c in range(NCORES):
        rows = slice(P * c, P * (c + 1))
        m = {}
        for t in ("q", "k", "v"):
            for part in ("r", "i"):
                m[f"x{t}_{part}"] = xs[t + part]
        for t, wr_name, wi_name in (("q", "wq_r", "wq_i"),
                                    ("k", "wk_r", "wk_i")):
            s = scale if t == "q" else np.float32(1.0)
            wr = np.asarray(inputs[wr_name], dtype=np.float32)[rows] * s
            wi = np.asarray(inputs[wi_name], dtype=np.float32)[rows] * s
            for h in range(HPC):
                hr = slice(DH * h, DH * (h + 1))
                if t == "q":
                    wa = np.concatenate([wr[hr].T, wi[hr].T], axis=1)
                    wb = np.concatenate([-wi[hr].T, wr[hr].T], axis=1)
                else:
                    wa = np.concatenate([wr[hr].T, -wi[hr].T], axis=1)
                    wb = np.concatenate([-wi[hr].T, -wr[hr].T], axis=1)
                m[f"w{t}_a{h}"] = _w_sbuf_layout(wa).astype(bf)
                m[f"w{t}_b{h}"] = _w_sbuf_layout(wb).astype(bf)
        # V^T weights, moving operand: [1024 d, 2 heads * (v_r 64 | v_i 64)]
        # chunked to [128, dc*256 + c]
        wvr = np.asarray(inputs["wv_r"], dtype=np.float32)[rows]  # [128,1024]
        wvi = np.asarray(inputs["wv_i"], dtype=np.float32)[rows]
        br = np.concatenate(
            [np.concatenate([wvr[DH * h:DH * (h + 1)].T,
                             wvi[DH * h:DH * (h + 1)].T], axis=1)
             for h in range(HPC)], axis=1)  # [1024, 256]
        bi = np.concatenate(
            [np.concatenate([-wvi[DH * h:DH * (h + 1)].T,
                             wvr[DH * h:DH * (h + 1)].T], axis=1)
             for h in range(HPC)], axis=1)
        m["wvt_r"] = np.ascontiguousarray(
            br.reshape(DC, P, 2 * P).transpose(1, 0, 2).reshape(
                P, DC * 2 * P)).astype(bf)
        m["wvt_i"] = np.ascontiguousarray(
            bi.reshape(DC, P, 2 * P).transpose(1, 0, 2).reshape(
                P, DC * 2 * P)).astype(bf)
        wo_r = np.asarray(inputs["wo_r"], dtype=np.float32)[:, rows]  # [D,128]
        wo_i = np.asarray(inputs["wo_i"], dtype=np.float32)[:, rows]
        m["wo_r"] = np.ascontiguousarray(wo_r.T).astype(bf)  # [128 d, 1024 m]
        m["wo_i"] = np.ascontiguousarray(wo_i.T).astype(bf)
        m["wo_in"] = np.ascontiguousarray(-wo_i.T).astype(bf)
        m["onesin"] = np.ones((P, P), dtype=bf)
        in_maps.append(m)
    return in_maps


LAST_RESULT = None


def _run(inputs, trace=False):
    global LAST_RESULT
    from concourse.bass_utils import run_bass_kernel_spmd
    if "nc" not in _CACHE:
        _CACHE["nc"] = _build()
    nc = _CACHE["nc"]
    in_maps = _prepare_in_maps(inputs)
    if trace:
        os.environ.pop("BASS_NEVER_TRACE", None)
    else:
        os.environ["BASS_NEVER_TRACE"] = "1"
    res = run_bass_kernel_spmd(nc, in_maps, core_ids=list(range(NCORES)),
                               trace=trace)
    LAST_RESULT = res
    # po rows gt*P+p, cols (2*mc+ri)*TBLK+tok
    acc = np.zeros((NT * P, 2 * DC * TBLK), np.float32)
    for c in range(NCORES):
        acc += res.results[c]["po"].astype(np.float32)

    t = acc.reshape(NT, P, DC, 2, TBLK)
    out = np.empty((B, S, D, 2), np.float32)
    for ri in range(2):
        # value at [gt, p, mc, ri, tok] = out_part[d=mc*128+p, gt*512+tok]
        comp = t[:, :, :, ri, :].transpose(2, 1, 0, 3).reshape(D, B * S)
        out[..., ri] = comp.T.reshape(B, S, D)
    return out


def kernel(**inputs):
    return _run(inputs, trace=False)


# revision 34
# speedup vs baseline: 1.1122x; 1.0046x over previous
"""ComplexMultiHeadAttention on 8 TRN2 NeuronCores (Bass/Tile) — fused stream.

Problem: B=4, S=1024, D_MODEL=1024, N_HEADS=16, D_HEAD=64, complex-valued
activations stored as a trailing dim of size 2 (real, imag).

    q = to_heads(complex_linear(queries, wq));  k, v likewise
    s_r + i*s_i = (q_r + i q_i)(k_r + i k_i)^T / sqrt(dh)
    a_r = softmax(s_r), a_i = softmax(s_i)      (independent softmaxes)
    o = complex_bmm(a, v);  out = complex_linear(concat_heads(o), wo)

Sharding: head-parallel. Core c owns heads {2c, 2c+1} = 128 contiguous dims
of the hidden axis. Weights row-sliced for QKV, wo column-sliced; the host
sums the 8 partial outputs — no on-device collectives.

Key design points (TRN2):
  - ONE fused tensor stream: attention(b) is interleaved, per key-chunk
    unit, with "filler" matmuls from oproj(b-1) and the q/k/v projections
    of (b+1).  Every engine's work is spread over the whole batch window,
    so no phase boundary ever idles the PE (which would also drop the
    DVFS p-state to half rate for ~5us).
  - V is projected directly in TRANSPOSED form: V^T = X^T W per 128-token
    block (X slice stationary, weights moving, 256-wide streams).  The
    value matrix lands token-major straight out of the PE — no DMA
    transposes, nothing on the scalar queue but the exps.
  - vb elimination: ob = va^T u_i and the epilogue reads its halves
    swapped (the complex cross terms only differ by that swap).
  - Z (softmax denominators): u chunks are accumulated with bf16 vector
    adds into u_acc per (head, q-half); ONE ones-matmul pair per group
    replaces 8 — Z tensor cols drop 8x and two PSUM banks are freed,
    which is exactly what lets scores/AV/aux all fit in 8 banks.
  - PSUM: scores 2x2 banks, AV wide 1x2 banks, aux (proj/oproj/Z) 2x1.
  - All matmuls bf16 (f32 PSUM accumulation); softmax over keys skips
    max-subtraction (scores are O(1) by construction).
"""

import os
import numpy as np
import ml_dtypes
from contextlib import ExitStack

import concourse.bass as bass
import concourse.tile as tile
from concourse import bacc, mybir

F32 = mybir.dt.float32
BF16 = mybir.dt.bfloat16
EXP = mybir.ActivationFunctionType.Exp

B, S, D, H, DH = 4, 1024, 1024, 16, 64
NCORES = 8
P = 128            # partitions / chunk size
TBLK = 512         # token block (matmul free dim)
WBLK = 2 * TBLK    # wide tile (2 psum banks)
DC = D // P        # 8 d-chunks
KC = S // P        # 8 key chunks per batch
HPC = H // NCORES  # 2 heads per core
NT = (B * S) // TBLK
NUNITS = HPC * 2 * KC  # 32 attention units per batch

_CACHE = {}


def _build():
    nc = bacc.Bacc("TRN2", target_bir_lowering=False, debug=False,
                   num_devices=NCORES)

    # partition-major tiled layout: row gt*128+p, col dc*512+tok
    x_ap = {}
    for t in ("q", "k", "v"):
        for part in ("r", "i"):
            x_ap[t + part] = nc.dram_tensor(
                f"x{t}_{part}", [NT * P, DC * TBLK],
                BF16, kind="ExternalInput").ap()
    w_ap = {}
    for t in ("q", "k"):
        for h in range(HPC):
            for suf in ("a", "b"):
                w_ap[f"{t}{suf}{h}"] = nc.dram_tensor(
                    f"w{t}_{suf}{h}", [P, D], BF16, kind="ExternalInput").ap()
    wvt_ap = {}
    for part in ("r", "i"):
        wvt_ap[part] = nc.dram_tensor(
            f"wvt_{part}", [P, DC * 2 * P], BF16, kind="ExternalInput").ap()
    wo_ap = {}
    for suf in ("r", "i", "in"):
        wo_ap[suf] = nc.dram_tensor(
            f"wo_{suf}", [P, D], BF16, kind="ExternalInput").ap()
    ones_ap = nc.dram_tensor("onesin", [P, P], BF16, kind="ExternalInput").ap()
    # output: row gt*128+p, col (2*mc+ri)*512+tok  (r/i interleaved per mc)
    po_ap = nc.dram_tensor("po", [NT * P, 2 * DC * TBLK], BF16,
                           kind="ExternalOutput").ap()

    with tile.TileContext(nc) as tc, ExitStack() as ctx:
        wpool = ctx.enter_context(tc.tile_pool(name="w", bufs=1))
        xpool = ctx.enter_context(tc.tile_pool(name="x", bufs=8))
        qkpool = ctx.enter_context(tc.tile_pool(name="qk", bufs=2))
        vpool = ctx.enter_context(tc.tile_pool(name="v", bufs=2))
        opool = ctx.enter_context(tc.tile_pool(name="ost", bufs=2))
        upool = ctx.enter_context(tc.tile_pool(name="u", bufs=3))
        uaccpool = ctx.enter_context(tc.tile_pool(name="uacc", bufs=2))
        zpool = ctx.enter_context(tc.tile_pool(name="z", bufs=2))
        tmppool = ctx.enter_context(tc.tile_pool(name="tmp", bufs=2))
        popool = ctx.enter_context(tc.tile_pool(name="po", bufs=2))
        # PSUM: scores 2x2 banks + AV wide 1x2 banks + aux 2x1 bank = 8
        sps = ctx.enter_context(tc.tile_pool(name="sp", bufs=2, space="PSUM"))
        avps = ctx.enter_context(tc.tile_pool(name="av", bufs=1,
                                              space="PSUM"))
        auxps = ctx.enter_context(tc.tile_pool(name="ax", bufs=2,
                                               space="PSUM"))

        # Startup DMAs: weights on the scalar HWDGE queue in consumption
        # order (wvt, wq, wk, wo, ones), x loads on the sync queue in
        # consumption order — the two queues stream from HBM in parallel
        # and each arrival matches its consumer.
        wt = {}
        wot = {}
        wvt = {}
        xtiles = {}

        def load_w(dst, key, ap, pfx):
            dst[key] = wpool.tile(list(ap.shape), BF16, tag=f"{pfx}_{key}",
                                  name=f"{pfx}_{key}")
            nc.scalar.dma_start(dst[key][:], ap[:])

        def emit_xloads(b, t):
            # r on the sync HWDGE queue, i on the scalar HWDGE queue: a
            # single DMA queue tops out near ~190 GB/s, two in parallel
            # keep up.  (gpsimd is SWDGE, ~55 GB/s — loads never go there.)
            for half in range(2):
                for part, q in (("r", nc.sync), ("i", nc.scalar)):
                    gt = 2 * b + half
                    xt = xpool.tile([P, DC * TBLK], BF16, tag="xt",
                                    name="xt")
                    q.dma_start(
                        xt[:], x_ap[t + part][gt * P:(gt + 1) * P, :])
                    xtiles[(b, t, part, half)] = xt

        # startup: interleave weight loads with batch-0 x loads so each
        # queue's arrival order matches proj(0)'s consumption order
        for part in ("r", "i"):
            load_w(wvt, part, wvt_ap[part], "wvt")
        emit_xloads(0, "v")
        for h in range(HPC):
            for suf in ("a", "b"):
                load_w(wt, f"q{suf}{h}", w_ap[f"q{suf}{h}"], "w")
        emit_xloads(0, "q")
        for h in range(HPC):
            for suf in ("a", "b"):
                load_w(wt, f"k{suf}{h}", w_ap[f"k{suf}{h}"], "w")
        emit_xloads(0, "k")
        for suf, ap in wo_ap.items():
            load_w(wot, suf, ap, "wo")
        ones = wpool.tile([P, P], BF16, tag="ones", name="ones")
        nc.scalar.dma_start(ones[:], ones_ap[:])

        def proj_gen(b, qcat, kcr, kci, va):
            """Projection of batch b as a stream of tensor quanta.

            Yields the emitted tensor-column count after each quantum.
            v first (V^T form), then q, then k; trailing vector ops of a
            psum group are emitted with its final quantum.
            """
            for half in range(2):
                xr = xtiles.pop((b, "v", "r", half))
                xi = xtiles.pop((b, "v", "i", half))
                for tb in range(4):
                    kc = half * 4 + tb
                    vps = auxps.tile([P, 2 * P], F32, tag="aux", name="vps")
                    for dcg in range(2):
                        for dc in range(dcg * 4, dcg * 4 + 4):
                            xs_ = slice(dc * TBLK + tb * P,
                                        dc * TBLK + (tb + 1) * P)
                            ws = slice(dc * 2 * P, (dc + 1) * 2 * P)
                            nc.tensor.matmul(
                                vps[:], xr[:, xs_], wvt["r"][:, ws],
                                start=(dc == 0), stop=False)
                            nc.tensor.matmul(
                                vps[:], xi[:, xs_], wvt["i"][:, ws],
                                start=False, stop=(dc == DC - 1))
                        if dcg == 1:
                            nc.vector.tensor_copy(
                                va[:, kc * 2 * P:(kc + 1) * 2 * P], vps[:])
                        yield 2048
            for t in ("q", "k"):
                for half in range(2):
                    xr = xtiles.pop((b, t, "r", half))
                    xi = xtiles.pop((b, t, "i", half))
                    hs = slice(half * TBLK, (half + 1) * TBLK)
                    for hh in range(2):
                        ps = auxps.tile([P, TBLK], F32, tag="aux",
                                        name="qkps")
                        wA = wt[f"{t}a{hh}"]
                        wB = wt[f"{t}b{hh}"]
                        for dcg in range(2):
                            for dc in range(dcg * 4, dcg * 4 + 4):
                                ws = slice(dc * P, (dc + 1) * P)
                                xs_ = slice(dc * TBLK, (dc + 1) * TBLK)
                                nc.tensor.matmul(
                                    ps[:], wA[:, ws], xr[:, xs_],
                                    start=(dc == 0), stop=False)
                                nc.tensor.matmul(
                                    ps[:], wB[:, ws], xi[:, xs_],
                                    start=False, stop=(dc == DC - 1))
                            if dcg == 1:
                                if t == "q":
                                    nc.vector.tensor_copy(qcat[hh][:, hs],
                                                          ps[:])
                                else:
                                    nc.vector.tensor_copy(kcr[hh][:, hs],
                                                          ps[:])
                                    nc.vector.tensor_scalar_mul(
                                        kci[hh][0:DH, hs], ps[DH:P, :], -1.0)
                                    nc.vector.tensor_copy(kci[hh][DH:P, hs],
                                                          ps[0:DH, :])
                            yield 4096

        def oproj_gen(b, o_stage, halves=(0, 1), pools=None, alt_copy=False):
            """O-projection of batch b as a stream of tensor quanta.

            po is stored in quarter chunks (gpsimd SWDGE) so the final
            store's DMA tail is short and earlier chunks overlap compute.
            `pools`: psum pools to rotate over and `alt_copy` alternates
            powide copies between vector and scalar (drain mode: nothing
            else runs, so doubling both resources halves the drain).
            """
            if pools is None:
                pools = ((auxps, "aux"),)
            pi_ = 0
            for half in halves:
                gt = 2 * b + half
                hs = slice(half * TBLK, (half + 1) * TBLK)
                powide = popool.tile([P, 2 * DC * TBLK], BF16, tag="pow",
                                     name="powide")
                for mc in range(DC):
                    ms = slice(mc * P, (mc + 1) * P)
                    for ri in range(2):
                        pool, ptag = pools[pi_ % len(pools)]
                        pi_ += 1
                        ps = pool.tile([P, TBLK], F32, tag=ptag,
                                       name="ops")
                        if ri == 0:
                            pairs = ((wot["r"], o_stage["r"]),
                                     (wot["in"], o_stage["i"]))
                        else:
                            pairs = ((wot["i"], o_stage["r"]),
                                     (wot["r"], o_stage["i"]))
                        nc.tensor.matmul(ps[:], pairs[0][0][:, ms],
                                         pairs[0][1][:, hs],
                                         start=True, stop=False)
                        nc.tensor.matmul(ps[:], pairs[1][0][:, ms],
                                         pairs[1][1][:, hs],
                                         start=False, stop=True)
                        c0 = (2 * mc + ri) * TBLK
                        if alt_copy and ri == 1:
                            nc.scalar.copy(powide[:, c0:c0 + TBLK], ps[:])
                        else:
                            nc.vector.tensor_copy(powide[:, c0:c0 + TBLK],
                                                  ps[:])
                        yield 1024
                    if mc % 2 == 1:
                        cs = slice((2 * mc - 2) * TBLK, (2 * mc + 2) * TBLK)
                        nc.gpsimd.dma_start(po_ap[gt * P:(gt + 1) * P, cs],
                                            powide[:, cs])

        def drain(gen):
            for _ in gen:
                pass

        def emit_window(b, qcat, kcr, kci, va, o_stage, filler,
                        mid_filler=None, xloader=None):
            """Attention units of batch b with filler interleaved.

            qb-outer unit order: both heads' q-half epilogues complete by
            mid-window, so `mid_filler` (last batch's own half-0 oproj)
            can be injected after unit 15.
            """
            units = [(h, qb, kc)
                     for qb in range(2) for h in range(HPC)
                     for kc in range(KC)]
            total_fill = (32768 if b >= 1 else 0) + \
                         (98304 if b + 1 < B else 0)
            per_unit = (total_fill + NUNITS - 1) // NUNITS
            swides = [None] * len(units)
            accs = {}
            budget = 0

            def emit_scores(n):
                h, qb, kc = units[n]
                qs = slice(qb * TBLK, (qb + 1) * TBLK)
                ks = slice(kc * P, (kc + 1) * P)
                sw = sps.tile([P, WBLK], F32, tag="sps", name="scorew")
                nc.tensor.matmul(sw[:, 0:TBLK], kcr[h][:, ks],
                                 qcat[h][:, qs], start=True, stop=True)
                nc.tensor.matmul(sw[:, TBLK:WBLK], kci[h][:, ks],
                                 qcat[h][:, qs], start=True, stop=True)
                swides[n] = sw

            def emit_epilogue(h, qb, uacc, avw):
                # Z = ones^T u_acc (one matmul pair per (h,qb)), then
                # o_r = (v_r.T u_r)/Z_r - (v_i.T u_i)/Z_i etc.
                # ob (= avw cols TBLK:) is va^T u_i, halves swapped in
                # the combine; psum+sbuf DVE inputs are exempt from
                # the same-base-partition rule.
                qs = slice(qb * TBLK, (qb + 1) * TBLK)
                zps_r = auxps.tile([P, TBLK], F32, tag="aux", name="zpr")
                nc.tensor.matmul(zps_r[:], ones[:], uacc[:, 0:TBLK],
                                 start=True, stop=True)
                zps_i = auxps.tile([P, TBLK], F32, tag="aux", name="zpi")
                nc.tensor.matmul(zps_i[:], ones[:], uacc[:, TBLK:WBLK],
                                 start=True, stop=True)
                zinv = zpool.tile([P, WBLK], F32, tag="zinv", name="zinv")
                nc.vector.reciprocal_approx_fast(zinv[:, 0:TBLK], zps_r[:])
                nc.vector.reciprocal_approx_fast(zinv[:, TBLK:WBLK],
                                                 zps_i[:])
                tmpa = tmppool.tile([P, TBLK], F32, tag="tmpa", name="tmpa")
                nc.vector.tensor_mul(tmpa[:], avw[:, 0:TBLK],
                                     zinv[:, 0:TBLK])
                tmpb = tmppool.tile([P, TBLK], F32, tag="tmpb", name="tmpb")
                nc.vector.tensor_mul(tmpb[0:DH, :], avw[DH:P, TBLK:WBLK],
                                     zinv[DH:P, TBLK:WBLK])
                nc.vector.tensor_mul(tmpb[DH:P, :], avw[0:DH, TBLK:WBLK],
                                     zinv[0:DH, TBLK:WBLK])
                dst = slice(DH * h, DH * (h + 1))
                nc.vector.tensor_sub(o_stage["r"][dst, qs],
                                     tmpa[0:DH, :], tmpb[0:DH, :])
                nc.vector.tensor_add(o_stage["i"][dst, qs],
                                     tmpa[DH:P, :], tmpb[DH:P, :])

            pending = None
            emit_scores(0)
            for n, (h, qb, kc) in enumerate(units):
                while xloader and xloader[0][0] <= n:
                    xloader.pop(0)[1]()
                if n + 1 < len(units):
                    emit_scores(n + 1)
                if n == NUNITS // 2 and mid_filler is not None:
                    filler.append(mid_filler)
                    per_unit += 1024
                first, last = kc == 0, kc == KC - 1
                if first:
                    uacc = uaccpool.tile([P, WBLK], BF16, tag="uacc",
                                         name="uacc")
                    avw = avps.tile([P, WBLK], F32, tag="av", name="avw")
                    accs[(h, qb)] = (uacc, avw)
                    u = uacc
                    nc.scalar.activation(uacc[:], swides[n][:], EXP)
                else:
                    uacc, avw = accs[(h, qb)]
                    u = upool.tile([P, WBLK], BF16, tag="u", name="u")
                    nc.scalar.activation(u[:], swides[n][:], EXP)
                    nc.vector.tensor_add(uacc[:], uacc[:], u[:])
                swides[n] = None
                # filler; kc==0 units get a bonus pop so the avw-reuse
                # boundary (previous group's deferred epilogue) is hidden
                budget += per_unit + (2048 if first else 0)
                while budget > 0 and filler:
                    try:
                        budget -= next(filler[0])
                    except StopIteration:
                        filler.pop(0)
                # previous group's Z + epilogue, deferred here (one unit of
                # extra slack for its exp -> u_acc add chain) but before
                # this AV so the single avw psum buffer frees in time
                if pending is not None:
                    emit_epilogue(*pending)
                    pending = None
                # AV
                vsl = va[:, kc * 2 * P + h * P:kc * 2 * P + (h + 1) * P]
                nc.tensor.matmul(avw[:, 0:TBLK], vsl, u[:, 0:TBLK],
                                 start=first, stop=last)
                nc.tensor.matmul(avw[:, TBLK:WBLK], vsl, u[:, TBLK:WBLK],
                                 start=first, stop=last)
                if last:
                    pending = (h, qb, uacc, avw)
                    del accs[(h, qb)]
            emit_epilogue(*pending)
            # drain leftover filler
            for g in filler:
                drain(g)

        # ---- pipelined emission: one continuous tensor stream ----
        stage = {}

        def new_stage(b):
            qcat = [qkpool.tile([P, S], BF16, tag=f"qcat{h}", name=f"qcat{h}")
                    for h in range(HPC)]
            kcr = [qkpool.tile([P, S], BF16, tag=f"kcr{h}", name=f"kcr{h}")
                   for h in range(HPC)]
            kci = [qkpool.tile([P, S], BF16, tag=f"kci{h}", name=f"kci{h}")
                   for h in range(HPC)]
            # va: [128 tokens-in-chunk, kc*256 + h*128 + [v_r(64)|v_i(64)]]
            va = vpool.tile([P, 2 * S], BF16, tag="va", name="va")
            o_stage = {p: opool.tile([P, S], BF16, tag=f"ost{p}",
                                     name=f"ost{p}")
                       for p in ("r", "i")}
            stage[b] = (qcat, kcr, kci, va, o_stage)

        new_stage(0)
        drain(proj_gen(0, *stage[0][:4]))
        for b in range(B):
            xloader = None
            if b + 1 < B:
                # v loads issue before the window; q/k issue mid-window so
                # their descriptor-trigger instructions don't delay the
                # scalar queue's first exps
                emit_xloads(b + 1, "v")
                xloader = [(6, lambda bb=b + 1: emit_xloads(bb, "q")),
                           (14, lambda bb=b + 1: emit_xloads(bb, "k"))]
                new_stage(b + 1)
            filler = []
            if b >= 1:
                filler.append(oproj_gen(b - 1, stage[b - 1][4]))
            if b + 1 < B:
                filler.append(proj_gen(b + 1, *stage[b + 1][:4]))
            # last window: inject this batch's own half-0 oproj once both
            # q-half-0 epilogues are in (qb-outer order, after unit 15)
            mid = (oproj_gen(b, stage[b][4], halves=(0,))
                   if b == B - 1 else None)
            emit_window(b, *stage[b], filler, mid_filler=mid,
                        xloader=xloader)
            if b >= 1:
                del stage[b - 1]
        drain(oproj_gen(B - 1, stage[B - 1][4], halves=(1,),
                        pools=((auxps, "aux"), (sps, "sps")),
                        alt_copy=True))

    nc.compile()
    return nc


def _w_sbuf_layout(w_t):
    """[D, 128] weight-transpose slice -> SBUF layout [128, dc*128+o]."""
    return np.ascontiguousarray(
        w_t.reshape(DC, P, P).transpose(1, 0, 2).reshape(P, D))


def _tile_x(xT, dtype):
    """[D, B*S] -> partition-major [NT*P, DC*TBLK] (row gt*P+p, col dc*TBLK+t)."""
    t = xT.reshape(DC, P, NT, TBLK).transpose(2, 1, 0, 3)
    return np.ascontiguousarray(t.reshape(NT * P, DC * TBLK)).astype(dtype)


def _prepare_in_maps(inputs):
    bf = ml_dtypes.bfloat16
    xs = {}
    for name, t in (("queries", "q"), ("keys", "k"), ("values", "v")):
        x = np.asarray(inputs[name], dtype=np.float32)  # [B,S,D,2]
        flat = x.reshape(B * S, D, 2)
        xs[t + "r"] = _tile_x(flat[:, :, 0].T, bf)
        xs[t + "i"] = _tile_x(flat[:, :, 1].T, bf)

    scale = np.float32(1.0 / np.sqrt(DH))
    in_maps = []
    for c in range(NCORES):
        rows = slice(P * c, P * (c + 1))
        m = {}
        for t in ("q", "k", "v"):
            for part in ("r", "i"):
                m[f"x{t}_{part}"] = xs[t + part]
        for t, wr_name, wi_name in (("q", "wq_r", "wq_i"),
                                    ("k", "wk_r", "wk_i")):
            s = scale if t == "q" else np.float32(1.0)
            wr = np.asarray(inputs[wr_name], dtype=np.float32)[rows] * s
            wi = np.asarray(inputs[wi_name], dtype=np.float32)[rows] * s
            for h in range(HPC):
                hr = slice(DH * h, DH * (h + 1))
                if t == "q":
                    wa = np.concatenate([wr[hr].T, wi[hr].T], axis=1)
                    wb = np.concatenate([-wi[hr].T, wr[hr].T], axis=1)
                else:
                    wa = np.concatenate([wr[hr].T, -wi[hr].T], axis=1)
                    wb = np.concatenate([-wi[hr].T, -wr[hr].T], axis=1)
                m[f"w{t}_a{h}"] = _w_sbuf_layout(wa).astype(bf)
                m[f"w{t}_b{h}"] = _w_sbuf_layout(wb).astype(bf)
        # V^T weights, moving operand: [1024 d, 2 heads * (v_r 64 | v_i 64)]
        # chunked to [128, dc*256 + c]
        wvr = np.asarray(inputs["wv_r"], dtype=np.float32)[rows]  # [128,1024]
        wvi = np.asarray(inputs["wv_i"], dtype=np.float32)[rows]
        br = np.concatenate(
            [np.concatenate([wvr[DH * h:DH * (h + 1)].T,
                             wvi[DH * h:DH * (h + 1)].T], axis=1)
             for h in range(HPC)], axis=1)  # [1024, 256]
        bi = np.concatenate(
            [np.concatenate([-wvi[DH * h:DH * (h + 1)].T,
                             wvr[DH * h:DH * (h + 1)].T], axis=1)
             for h in range(HPC)], axis=1)
        m["wvt_r"] = np.ascontiguousarray(
            br.reshape(DC, P, 2 * P).transpose(1, 0, 2).reshape(
                P, DC * 2 * P)).astype(bf)
        m["wvt_i"] = np.ascontiguousarray(
            bi.reshape(DC, P, 2 * P).transpose(1, 0, 2).reshape(
                P, DC * 2 * P)).astype(bf)
        wo_r = np.asarray(inputs["wo_r"], dtype=np.float32)[:, rows]  # [D,128]
        wo_i = np.asarray(inputs["wo_i"], dtype=np.float32)[:, rows]
        m["wo_r"] = np.ascontiguousarray(wo_r.T).astype(bf)  # [128 d, 1024 m]
        m["wo_i"] = np.ascontiguousarray(wo_i.T).astype(bf)
        m["wo_in"] = np.ascontiguousarray(-wo_i.T).astype(bf)
        m["onesin"] = np.ones((P, P), dtype=bf)
        in_maps.append(m)
    return in_maps


LAST_RESULT = None


def _run(inputs, trace=False):
    global LAST_RESULT
    from concourse.bass_utils import run_bass_kernel_spmd
    if "nc" not in _CACHE:
        _CACHE["nc"] = _build()
    nc = _CACHE["nc"]
    in_maps = _prepare_in_maps(inputs)
    if trace:
        os.environ.pop("BASS_NEVER_TRACE", None)
    else:
        os.environ["BASS_NEVER_TRACE"] = "1"
    res = run_bass_kernel_spmd(nc, in_maps, core_ids=list(range(NCORES)),
                               trace=trace)
    LAST_RESULT = res
    # po rows gt*P+p, cols (2*mc+ri)*TBLK+tok
    acc = np.zeros((NT * P, 2 * DC * TBLK), np.float32)
    for c in range(NCORES):
        acc += res.results[c]["po"].astype(np.float32)

    t = acc.reshape(NT, P, DC, 2, TBLK)
    out = np.empty((B, S, D, 2), np.float32)
    for ri in range(2):
        # value at [gt, p, mc, ri, tok] = out_part[d=mc*128+p, gt*512+tok]
        comp = t[:, :, :, ri, :].transpose(2, 1, 0, 3).reshape(D, B * S)
        out[..., ri] = comp.T.reshape(B, S, D)
    return out


def kernel(**inputs):
    return _run(inputs, trace=False)


# revision 37
# speedup vs baseline: 1.1402x; 1.0252x over previous
"""ComplexMultiHeadAttention on 8 TRN2 NeuronCores (Bass/Tile) — fused stream.

Problem: B=4, S=1024, D_MODEL=1024, N_HEADS=16, D_HEAD=64, complex-valued
activations stored as a trailing dim of size 2 (real, imag).

    q = to_heads(complex_linear(queries, wq));  k, v likewise
    s_r + i*s_i = (q_r + i q_i)(k_r + i k_i)^T / sqrt(dh)
    a_r = softmax(s_r), a_i = softmax(s_i)      (independent softmaxes)
    o = complex_bmm(a, v);  out = complex_linear(concat_heads(o), wo)

Sharding: head-parallel. Core c owns heads {2c, 2c+1} = 128 contiguous dims
of the hidden axis. Weights row-sliced for QKV, wo column-sliced; the host
sums the 8 partial outputs — no on-device collectives.

Key design points (TRN2):
  - ONE fused tensor stream: attention(b) is interleaved, per key-chunk
    unit, with "filler" matmuls from oproj(b-1) and the q/k/v projections
    of (b+1).  Every engine's work is spread over the whole batch window,
    so no phase boundary ever idles the PE (which would also drop the
    DVFS p-state to half rate for ~5us).
  - V is projected directly in TRANSPOSED form: V^T = X^T W per 128-token
    block (X slice stationary, weights moving, 256-wide streams).  The
    value matrix lands token-major straight out of the PE — no DMA
    transposes, nothing on the scalar queue but the exps.
  - vb elimination: ob = va^T u_i and the epilogue reads its halves
    swapped (the complex cross terms only differ by that swap).
  - Z (softmax denominators): u chunks are accumulated with bf16 vector
    adds into u_acc per (head, q-half); ONE ones-matmul pair per group
    replaces 8 — Z tensor cols drop 8x and two PSUM banks are freed,
    which is exactly what lets scores/AV/aux all fit in 8 banks.
  - PSUM: scores 2x2 banks, AV wide 1x2 banks, aux (proj/oproj/Z) 2x1.
  - All matmuls bf16 (f32 PSUM accumulation); softmax over keys skips
    max-subtraction (scores are O(1) by construction).
"""

import os
import numpy as np
import ml_dtypes
from contextlib import ExitStack

import concourse.bass as bass
import concourse.tile as tile
from concourse import bacc, mybir

F32 = mybir.dt.float32
BF16 = mybir.dt.bfloat16
EXP = mybir.ActivationFunctionType.Exp

B, S, D, H, DH = 4, 1024, 1024, 16, 64
NCORES = 8
P = 128            # partitions / chunk size
TBLK = 512         # token block (matmul free dim)
WBLK = 2 * TBLK    # wide tile (2 psum banks)
DC = D // P        # 8 d-chunks
KC = S // P        # 8 key chunks per batch
HPC = H // NCORES  # 2 heads per core
NT = (B * S) // TBLK
NUNITS = HPC * 2 * KC  # 32 attention units per batch

_CACHE = {}


def _build():
    nc = bacc.Bacc("TRN2", target_bir_lowering=False, debug=False,
                   num_devices=NCORES)

    # partition-major tiled layout: row gt*128+p, col dc*512+tok
    x_ap = {}
    for t in ("q", "k", "v"):
        for part in ("r", "i"):
            x_ap[t + part] = nc.dram_tensor(
                f"x{t}_{part}", [NT * P, DC * TBLK],
                BF16, kind="ExternalInput").ap()
    w_ap = {}
    for t in ("q", "k"):
        for h in range(HPC):
            for suf in ("a", "b"):
                w_ap[f"{t}{suf}{h}"] = nc.dram_tensor(
                    f"w{t}_{suf}{h}", [P, D], BF16, kind="ExternalInput").ap()
    wvt_ap = {}
    for part in ("r", "i"):
        wvt_ap[part] = nc.dram_tensor(
            f"wvt_{part}", [P, DC * 2 * P], BF16, kind="ExternalInput").ap()
    wo_ap = {}
    for suf in ("r", "i", "in"):
        wo_ap[suf] = nc.dram_tensor(
            f"wo_{suf}", [P, D], BF16, kind="ExternalInput").ap()
    ones_ap = nc.dram_tensor("onesin", [P, P], BF16, kind="ExternalInput").ap()
    # output: row gt*128+p, col (2*mc+ri)*512+tok  (r/i interleaved per mc)
    po_ap = nc.dram_tensor("po", [NT * P, 2 * DC * TBLK], BF16,
                           kind="ExternalOutput").ap()

    with tile.TileContext(nc) as tc, ExitStack() as ctx:
        wpool = ctx.enter_context(tc.tile_pool(name="w", bufs=1))
        xpool = ctx.enter_context(tc.tile_pool(name="x", bufs=8))
        qkpool = ctx.enter_context(tc.tile_pool(name="qk", bufs=2))
        vpool = ctx.enter_context(tc.tile_pool(name="v", bufs=2))
        opool = ctx.enter_context(tc.tile_pool(name="ost", bufs=2))
        upool = ctx.enter_context(tc.tile_pool(name="u", bufs=3))
        uaccpool = ctx.enter_context(tc.tile_pool(name="uacc", bufs=2))
        zpool = ctx.enter_context(tc.tile_pool(name="z", bufs=2))
        tmppool = ctx.enter_context(tc.tile_pool(name="tmp", bufs=2))
        popool = ctx.enter_context(tc.tile_pool(name="po", bufs=2))
        # PSUM: scores 2x2 banks + AV wide 1x2 banks + aux 2x1 bank = 8
        sps = ctx.enter_context(tc.tile_pool(name="sp", bufs=2, space="PSUM"))
        avps = ctx.enter_context(tc.tile_pool(name="av", bufs=1,
                                              space="PSUM"))
        auxps = ctx.enter_context(tc.tile_pool(name="ax", bufs=2,
                                               space="PSUM"))

        # Startup DMAs: weights on the scalar HWDGE queue in consumption
        # order (wvt, wq, wk, wo, ones), x loads on the sync queue in
        # consumption order — the two queues stream from HBM in parallel
        # and each arrival matches its consumer.
        wt = {}
        wot = {}
        wvt = {}
        xtiles = {}

        def load_w(dst, key, ap, pfx):
            dst[key] = wpool.tile(list(ap.shape), BF16, tag=f"{pfx}_{key}",
                                  name=f"{pfx}_{key}")
            nc.scalar.dma_start(dst[key][:], ap[:])

        def emit_xloads(b, t):
            # all x on the sync HWDGE queue in consumption order; weights
            # stream on the scalar queue in parallel.  (gpsimd is SWDGE,
            # ~55 GB/s — loads never go there.)
            for half in range(2):
                for part in ("r", "i"):
                    gt = 2 * b + half
                    xt = xpool.tile([P, DC * TBLK], BF16, tag="xt",
                                    name="xt")
                    nc.sync.dma_start(
                        xt[:], x_ap[t + part][gt * P:(gt + 1) * P, :])
                    xtiles[(b, t, part, half)] = xt

        # startup: interleave weight loads with batch-0 x loads so each
        # queue's arrival order matches proj(0)'s consumption order
        for part in ("r", "i"):
            load_w(wvt, part, wvt_ap[part], "wvt")
        emit_xloads(0, "v")
        for h in range(HPC):
            for suf in ("a", "b"):
                load_w(wt, f"q{suf}{h}", w_ap[f"q{suf}{h}"], "w")
        emit_xloads(0, "q")
        for h in range(HPC):
            for suf in ("a", "b"):
                load_w(wt, f"k{suf}{h}", w_ap[f"k{suf}{h}"], "w")
        emit_xloads(0, "k")
        for suf, ap in wo_ap.items():
            load_w(wot, suf, ap, "wo")
        ones = wpool.tile([P, P], BF16, tag="ones", name="ones")
        nc.scalar.dma_start(ones[:], ones_ap[:])

        def proj_gen(b, qcat, kcr, kci, va):
            """Projection of batch b as a stream of tensor quanta.

            Yields the emitted tensor-column count after each quantum.
            v first (V^T form), then q, then k; trailing vector ops of a
            psum group are emitted with its final quantum.
            """
            for half in range(2):
                xr = xtiles.pop((b, "v", "r", half))
                xi = xtiles.pop((b, "v", "i", half))
                for tb in range(4):
                    kc = half * 4 + tb
                    vps = auxps.tile([P, 2 * P], F32, tag="aux", name="vps")
                    for dcg in range(2):
                        for dc in range(dcg * 4, dcg * 4 + 4):
                            xs_ = slice(dc * TBLK + tb * P,
                                        dc * TBLK + (tb + 1) * P)
                            ws = slice(dc * 2 * P, (dc + 1) * 2 * P)
                            nc.tensor.matmul(
                                vps[:], xr[:, xs_], wvt["r"][:, ws],
                                start=(dc == 0), stop=False)
                            nc.tensor.matmul(
                                vps[:], xi[:, xs_], wvt["i"][:, ws],
                                start=False, stop=(dc == DC - 1))
                        if dcg == 1:
                            nc.vector.tensor_copy(
                                va[:, kc * 2 * P:(kc + 1) * 2 * P], vps[:])
                        yield 2048
            for t in ("q", "k"):
                for half in range(2):
                    xr = xtiles.pop((b, t, "r", half))
                    xi = xtiles.pop((b, t, "i", half))
                    hs = slice(half * TBLK, (half + 1) * TBLK)
                    for hh in range(2):
                        ps = auxps.tile([P, TBLK], F32, tag="aux",
                                        name="qkps")
                        wA = wt[f"{t}a{hh}"]
                        wB = wt[f"{t}b{hh}"]
                        for dcg in range(2):
                            for dc in range(dcg * 4, dcg * 4 + 4):
                                ws = slice(dc * P, (dc + 1) * P)
                                xs_ = slice(dc * TBLK, (dc + 1) * TBLK)
                                nc.tensor.matmul(
                                    ps[:], wA[:, ws], xr[:, xs_],
                                    start=(dc == 0), stop=False)
                                nc.tensor.matmul(
                                    ps[:], wB[:, ws], xi[:, xs_],
                                    start=False, stop=(dc == DC - 1))
                            if dcg == 1:
                                if t == "q":
                                    nc.vector.tensor_copy(qcat[hh][:, hs],
                                                          ps[:])
                                else:
                                    nc.vector.tensor_copy(kcr[hh][:, hs],
                                                          ps[:])
                                    nc.vector.tensor_scalar_mul(
                                        kci[hh][0:DH, hs], ps[DH:P, :], -1.0)
                                    nc.vector.tensor_copy(kci[hh][DH:P, hs],
                                                          ps[0:DH, :])
                            yield 4096

        def oproj_gen(b, o_stage, halves=(0, 1), pools=None, alt_copy=False):
            """O-projection of batch b as a stream of tensor quanta.

            po is stored in quarter chunks (gpsimd SWDGE) so the final
            store's DMA tail is short and earlier chunks overlap compute.
            `pools`: psum pools to rotate over and `alt_copy` alternates
            powide copies between vector and scalar (drain mode: nothing
            else runs, so doubling both resources halves the drain).
            """
            if pools is None:
                pools = ((auxps, "aux"),)
            pi_ = 0
            for half in halves:
                gt = 2 * b + half
                hs = slice(half * TBLK, (half + 1) * TBLK)
                powide = popool.tile([P, 2 * DC * TBLK], BF16, tag="pow",
                                     name="powide")
                for mc in range(DC):
                    ms = slice(mc * P, (mc + 1) * P)
                    for ri in range(2):
                        pool, ptag = pools[pi_ % len(pools)]
                        pi_ += 1
                        ps = pool.tile([P, TBLK], F32, tag=ptag,
                                       name="ops")
                        if ri == 0:
                            pairs = ((wot["r"], o_stage["r"]),
                                     (wot["in"], o_stage["i"]))
                        else:
                            pairs = ((wot["i"], o_stage["r"]),
                                     (wot["r"], o_stage["i"]))
                        nc.tensor.matmul(ps[:], pairs[0][0][:, ms],
                                         pairs[0][1][:, hs],
                                         start=True, stop=False)
                        nc.tensor.matmul(ps[:], pairs[1][0][:, ms],
                                         pairs[1][1][:, hs],
                                         start=False, stop=True)
                        c0 = (2 * mc + ri) * TBLK
                        if alt_copy and ri == 1:
                            nc.scalar.copy(powide[:, c0:c0 + TBLK], ps[:])
                        else:
                            nc.vector.tensor_copy(powide[:, c0:c0 + TBLK],
                                                  ps[:])
                        yield 1024
                    if mc % 2 == 1:
                        # store on the sync HWDGE queue — gpsimd SWDGE runs
                        # at ~55 GB/s and its final chunk would be the tail
                        cs = slice((2 * mc - 2) * TBLK, (2 * mc + 2) * TBLK)
                        nc.sync.dma_start(po_ap[gt * P:(gt + 1) * P, cs],
                                          powide[:, cs])

        def drain(gen):
            for _ in gen:
                pass

        def emit_window(b, qcat, kcr, kci, va, o_stage, filler,
                        mid_filler=None, xloader=None):
            """Attention units of batch b with filler interleaved.

            qb-outer unit order: both heads' q-half epilogues complete by
            mid-window, so `mid_filler` (last batch's own half-0 oproj)
            can be injected after unit 15.
            """
            units = [(h, qb, kc)
                     for qb in range(2) for h in range(HPC)
                     for kc in range(KC)]
            total_fill = (32768 if b >= 1 else 0) + \
                         (98304 if b + 1 < B else 0)
            per_unit = (total_fill + NUNITS - 1) // NUNITS
            swides = [None] * len(units)
            accs = {}
            budget = 0

            def emit_scores(n):
                h, qb, kc = units[n]
                qs = slice(qb * TBLK, (qb + 1) * TBLK)
                ks = slice(kc * P, (kc + 1) * P)
                sw = sps.tile([P, WBLK], F32, tag="sps", name="scorew")
                nc.tensor.matmul(sw[:, 0:TBLK], kcr[h][:, ks],
                                 qcat[h][:, qs], start=True, stop=True)
                nc.tensor.matmul(sw[:, TBLK:WBLK], kci[h][:, ks],
                                 qcat[h][:, qs], start=True, stop=True)
                swides[n] = sw

            def emit_epilogue(h, qb, uacc, avw):
                # Z = ones^T u_acc (one matmul pair per (h,qb)), then
                # o_r = (v_r.T u_r)/Z_r - (v_i.T u_i)/Z_i etc.
                # ob (= avw cols TBLK:) is va^T u_i, halves swapped in
                # the combine; psum+sbuf DVE inputs are exempt from
                # the same-base-partition rule.
                qs = slice(qb * TBLK, (qb + 1) * TBLK)
                zps_r = auxps.tile([P, TBLK], F32, tag="aux", name="zpr")
                nc.tensor.matmul(zps_r[:], ones[:], uacc[:, 0:TBLK],
                                 start=True, stop=True)
                zps_i = auxps.tile([P, TBLK], F32, tag="aux", name="zpi")
                nc.tensor.matmul(zps_i[:], ones[:], uacc[:, TBLK:WBLK],
                                 start=True, stop=True)
                zinv = zpool.tile([P, WBLK], F32, tag="zinv", name="zinv")
                nc.vector.reciprocal_approx_fast(zinv[:, 0:TBLK], zps_r[:])
                nc.vector.reciprocal_approx_fast(zinv[:, TBLK:WBLK],
                                                 zps_i[:])
                tmpa = tmppool.tile([P, TBLK], F32, tag="tmpa", name="tmpa")
                nc.vector.tensor_mul(tmpa[:], avw[:, 0:TBLK],
                                     zinv[:, 0:TBLK])
                tmpb = tmppool.tile([P, TBLK], F32, tag="tmpb", name="tmpb")
                nc.vector.tensor_mul(tmpb[0:DH, :], avw[DH:P, TBLK:WBLK],
                                     zinv[DH:P, TBLK:WBLK])
                nc.vector.tensor_mul(tmpb[DH:P, :], avw[0:DH, TBLK:WBLK],
                                     zinv[0:DH, TBLK:WBLK])
                dst = slice(DH * h, DH * (h + 1))
                nc.vector.tensor_sub(o_stage["r"][dst, qs],
                                     tmpa[0:DH, :], tmpb[0:DH, :])
                nc.vector.tensor_add(o_stage["i"][dst, qs],
                                     tmpa[DH:P, :], tmpb[DH:P, :])

            pending = None
            emit_scores(0)
            for n, (h, qb, kc) in enumerate(units):
                while xloader and xloader[0][0] <= n:
                    xloader.pop(0)[1]()
                if n + 1 < len(units):
                    emit_scores(n + 1)
                if n == NUNITS // 2 and mid_filler is not None:
                    filler.append(mid_filler)
                    per_unit += 1024
                first, last = kc == 0, kc == KC - 1
                if first:
                    uacc = uaccpool.tile([P, WBLK], BF16, tag="uacc",
                                         name="uacc")
                    avw = avps.tile([P, WBLK], F32, tag="av", name="avw")
                    accs[(h, qb)] = (uacc, avw)
                    u = uacc
                    nc.scalar.activation(uacc[:], swides[n][:], EXP)
                else:
                    uacc, avw = accs[(h, qb)]
                    u = upool.tile([P, WBLK], BF16, tag="u", name="u")
                    nc.scalar.activation(u[:], swides[n][:], EXP)
                    nc.vector.tensor_add(uacc[:], uacc[:], u[:])
                swides[n] = None
                # filler; kc==0 units get a bonus pop so the avw-reuse
                # boundary (previous group's deferred epilogue) is hidden
                budget += per_unit + (2048 if first else 0)
                while budget > 0 and filler:
                    try:
                        budget -= next(filler[0])
                    except StopIteration:
                        filler.pop(0)
                # previous group's Z + epilogue, deferred here (one unit of
                # extra slack for its exp -> u_acc add chain) but before
                # this AV so the single avw psum buffer frees in time
                if pending is not None:
                    emit_epilogue(*pending)
                    pending = None
                # AV
                vsl = va[:, kc * 2 * P + h * P:kc * 2 * P + (h + 1) * P]
                nc.tensor.matmul(avw[:, 0:TBLK], vsl, u[:, 0:TBLK],
                                 start=first, stop=last)
                nc.tensor.matmul(avw[:, TBLK:WBLK], vsl, u[:, TBLK:WBLK],
                                 start=first, stop=last)
                if last:
                    pending = (h, qb, uacc, avw)
                    del accs[(h, qb)]
            emit_epilogue(*pending)
            # drain leftover filler
            for g in filler:
                drain(g)

        # ---- pipelined emission: one continuous tensor stream ----
        stage = {}

        def new_stage(b):
            qcat = [qkpool.tile([P, S], BF16, tag=f"qcat{h}", name=f"qcat{h}")
                    for h in range(HPC)]
            kcr = [qkpool.tile([P, S], BF16, tag=f"kcr{h}", name=f"kcr{h}")
                   for h in range(HPC)]
            kci = [qkpool.tile([P, S], BF16, tag=f"kci{h}", name=f"kci{h}")
                   for h in range(HPC)]
            # va: [128 tokens-in-chunk, kc*256 + h*128 + [v_r(64)|v_i(64)]]
            va = vpool.tile([P, 2 * S], BF16, tag="va", name="va")
            o_stage = {p: opool.tile([P, S], BF16, tag=f"ost{p}",
                                     name=f"ost{p}")
                       for p in ("r", "i")}
            stage[b] = (qcat, kcr, kci, va, o_stage)

        new_stage(0)
        drain(proj_gen(0, *stage[0][:4]))
        for b in range(B):
            xloader = None
            if b + 1 < B:
                # v loads issue before the window; q/k issue mid-window so
                # their descriptor-trigger instructions don't delay the
                # scalar queue's first exps
                emit_xloads(b + 1, "v")
                xloader = [(6, lambda bb=b + 1: emit_xloads(bb, "q")),
                           (14, lambda bb=b + 1: emit_xloads(bb, "k"))]
                new_stage(b + 1)
            filler = []
            if b >= 1:
                filler.append(oproj_gen(b - 1, stage[b - 1][4]))
            if b + 1 < B:
                filler.append(proj_gen(b + 1, *stage[b + 1][:4]))
            # last window: inject this batch's own half-0 oproj once both
            # q-half-0 epilogues are in (qb-outer order, after unit 15)
            mid = (oproj_gen(b, stage[b][4], halves=(0,), alt_copy=True)
                   if b == B - 1 else None)
            emit_window(b, *stage[b], filler, mid_filler=mid,
                        xloader=xloader)
            if b >= 1:
                del stage[b - 1]
        drain(oproj_gen(B - 1, stage[B - 1][4], halves=(1,),
                        pools=((auxps, "aux"), (sps, "sps")),
                        alt_copy=True))

    nc.compile()
    return nc


def _w_sbuf_layout(w_t):
    """[D, 128] weight-transpose slice -> SBUF layout [128, dc*128+o]."""
    return np.ascontiguousarray(
        w_t.reshape(DC, P, P).transpose(1, 0, 2).reshape(P, D))


def _tile_x(xT, dtype):
    """[D, B*S] -> partition-major [NT*P, DC*TBLK] (row gt*P+p, col dc*TBLK+t)."""
    t = xT.reshape(DC, P, NT, TBLK).transpose(2, 1, 0, 3)
    return np.ascontiguousarray(t.reshape(NT * P, DC * TBLK)).astype(dtype)


def _prepare_in_maps(inputs):
    bf = ml_dtypes.bfloat16
    xs = {}
    for name, t in (("queries", "q"), ("keys", "k"), ("values", "v")):
        x = np.asarray(inputs[name], dtype=np.float32)  # [B,S,D,2]
        flat = x.reshape(B * S, D, 2)
        xs[t + "r"] = _tile_x(flat[:, :, 0].T, bf)
        xs[t + "i"] = _tile_x(flat[:, :, 1].T, bf)

    scale = np.float32(1.0 / np.sqrt(DH))
    in_maps = []
    for c in range(NCORES):
        rows = slice(P * c, P * (c + 1))
        m = {}
        for t in ("q", "k", "v"):
            for part in ("r", "i"):
                m[f"x{t}_{part}"] = xs[t + part]
        for t, wr_name, wi_name in (("q", "wq_r", "wq_i"),
                                    ("k", "wk_r", "wk_i")):
            s = scale if t == "q" else np.float32(1.0)
            wr = np.asarray(inputs[wr_name], dtype=np.float32)[rows] * s
            wi = np.asarray(inputs[wi_name], dtype=np.float32)[rows] * s
            for h in range(HPC):
                hr = slice(DH * h, DH * (h + 1))
                if t == "q":
                    wa = np.concatenate([wr[hr].T, wi[hr].T], axis=1)
                    wb = np.concatenate([-wi[hr].T, wr[hr].T], axis=1)
                else:
                    wa = np.concatenate([wr[hr].T, -wi[hr].T], axis=1)
                    wb = np.concatenate([-wi[hr].T, -wr[hr].T], axis=1)
                m[f"w{t}_a{h}"] = _w_sbuf_layout(wa).astype(bf)
                m[f"w{t}_b{h}"] = _w_sbuf_layout(wb).astype(bf)
        # V^T weights, moving operand: [1024 d, 2 heads * (v_r 64 | v_i 64)]
        # chunked to [128, dc*256 + c]
        wvr = np.asarray(inputs["wv_r"], dtype=np.float32)[rows]  # [128,1024]
        wvi = np.asarray(inputs["wv_i"], dtype=np.float32)[rows]
        br = np.concatenate(
            [np.concatenate([wvr[DH * h:DH * (h + 1)].T,
                             wvi[DH * h:DH * (h + 1)].T], axis=1)
             for h in range(HPC)], axis=1)  # [1024, 256]
        bi = np.concatenate(
            [np.concatenate([-wvi[DH * h:DH * (h + 1)].T,
                             wvr[DH * h:DH * (h + 1)].T], axis=1)
             for h in range(HPC)], axis=1)
        m["wvt_r"] = np.ascontiguousarray(
            br.reshape(DC, P, 2 * P).transpose(1, 0, 2).reshape(
                P, DC * 2 * P)).astype(bf)
        m["wvt_i"] = np.ascontiguousarray(
            bi.reshape(DC, P, 2 * P).transpose(1, 0, 2).reshape(
                P, DC * 2 * P)).astype(bf)
        wo_r = np.asarray(inputs["wo_r"], dtype=np.float32)[:, rows]  # [D,128]
        wo_i = np.asarray(inputs["wo_i"], dtype=np.float32)[:, rows]
        m["wo_r"] = np.ascontiguousarray(wo_r.T).astype(bf)  # [128 d, 1024 m]
        m["wo_i"] = np.ascontiguousarray(wo_i.T).astype(bf)
        m["wo_in"] = np.ascontiguousarray(-wo_i.T).astype(bf)
        m["onesin"] = np.ones((P, P), dtype=bf)
        in_maps.append(m)
    return in_maps


LAST_RESULT = None


def _run(inputs, trace=False):
    global LAST_RESULT
    from concourse.bass_utils import run_bass_kernel_spmd
    if "nc" not in _CACHE:
        _CACHE["nc"] = _build()
    nc = _CACHE["nc"]
    in_maps = _prepare_in_maps(inputs)
    if trace:
        os.environ.pop("BASS_NEVER_TRACE", None)
    else:
        os.environ["BASS_NEVER_TRACE"] = "1"
    res = run_bass_kernel_spmd(nc, in_maps, core_ids=list(range(NCORES)),
                               trace=trace)
    LAST_RESULT = res
    # po rows gt*P+p, cols (2*mc+ri)*TBLK+tok
    acc = np.zeros((NT * P, 2 * DC * TBLK), np.float32)
    for c in range(NCORES):
        acc += res.results[c]["po"].astype(np.float32)

    t = acc.reshape(NT, P, DC, 2, TBLK)
    out = np.empty((B, S, D, 2), np.float32)
    for ri in range(2):
        # value at [gt, p, mc, ri, tok] = out_part[d=mc*128+p, gt*512+tok]
        comp = t[:, :, :, ri, :].transpose(2, 1, 0, 3).reshape(D, B * S)
        out[..., ri] = comp.T.reshape(B, S, D)
    return out


def kernel(**inputs):
    return _run(inputs, trace=False)


# revision 48
# speedup vs baseline: 1.1616x; 1.0188x over previous
"""ComplexMultiHeadAttention on 8 TRN2 NeuronCores (Bass/Tile) — fused stream.

Problem: B=4, S=1024, D_MODEL=1024, N_HEADS=16, D_HEAD=64, complex-valued
activations stored as a trailing dim of size 2 (real, imag).

    q = to_heads(complex_linear(queries, wq));  k, v likewise
    s_r + i*s_i = (q_r + i q_i)(k_r + i k_i)^T / sqrt(dh)
    a_r = softmax(s_r), a_i = softmax(s_i)      (independent softmaxes)
    o = complex_bmm(a, v);  out = complex_linear(concat_heads(o), wo)

Sharding: head-parallel. Core c owns heads {2c, 2c+1} = 128 contiguous dims
of the hidden axis. Weights row-sliced for QKV, wo column-sliced; the host
sums the 8 partial outputs — no on-device collectives.

Key design points (TRN2):
  - ONE fused tensor stream: attention(b) is interleaved, per key-chunk
    unit, with "filler" matmuls from oproj(b-1) and the q/k/v projections
    of (b+1).  Every engine's work is spread over the whole batch window,
    so no phase boundary ever idles the PE (which would also drop the
    DVFS p-state to half rate for ~5us).
  - V is projected directly in TRANSPOSED form: V^T = X^T W per 128-token
    block (X slice stationary, weights moving, 256-wide streams).  The
    value matrix lands token-major straight out of the PE — no DMA
    transposes, nothing on the scalar queue but the exps.
  - vb elimination: ob = va^T u_i and the epilogue reads its halves
    swapped (the complex cross terms only differ by that swap).
  - Z (softmax denominators): u chunks are accumulated with bf16 vector
    adds into u_acc per (head, q-half); ONE ones-matmul pair per group
    replaces 8 — Z tensor cols drop 8x and two PSUM banks are freed,
    which is exactly what lets scores/AV/aux all fit in 8 banks.
  - PSUM: scores 2x2 banks, AV wide 1x2 banks, aux (proj/oproj/Z) 2x1.
  - All matmuls bf16 (f32 PSUM accumulation); softmax over keys skips
    max-subtraction (scores are O(1) by construction).
"""

import os
import numpy as np
import ml_dtypes
from contextlib import ExitStack

import concourse.bass as bass
import concourse.tile as tile
from concourse import bacc, mybir

F32 = mybir.dt.float32
BF16 = mybir.dt.bfloat16
EXP = mybir.ActivationFunctionType.Exp

B, S, D, H, DH = 4, 1024, 1024, 16, 64
NCORES = 8
P = 128            # partitions / chunk size
TBLK = 512         # token block (matmul free dim)
WBLK = 2 * TBLK    # wide tile (2 psum banks)
DC = D // P        # 8 d-chunks
KC = S // P        # 8 key chunks per batch
HPC = H // NCORES  # 2 heads per core
NT = (B * S) // TBLK
NUNITS = HPC * 2 * KC  # 32 attention units per batch

_CACHE = {}


def _build():
    nc = bacc.Bacc("TRN2", target_bir_lowering=False, debug=False,
                   num_devices=NCORES)

    # partition-major tiled layout: row gt*128+p, col dc*512+tok
    x_ap = {}
    for t in ("q", "k", "v"):
        for part in ("r", "i"):
            x_ap[t + part] = nc.dram_tensor(
                f"x{t}_{part}", [NT * P, DC * TBLK],
                BF16, kind="ExternalInput").ap()
    w_ap = {}
    for t in ("q", "k"):
        for h in range(HPC):
            for suf in ("a", "b"):
                w_ap[f"{t}{suf}{h}"] = nc.dram_tensor(
                    f"w{t}_{suf}{h}", [P, D], BF16, kind="ExternalInput").ap()
    wvt_ap = {}
    for part in ("r", "i"):
        wvt_ap[part] = nc.dram_tensor(
            f"wvt_{part}", [P, DC * 2 * P], BF16, kind="ExternalInput").ap()
    wo_ap = {}
    for suf in ("r", "i", "in"):
        wo_ap[suf] = nc.dram_tensor(
            f"wo_{suf}", [P, D], BF16, kind="ExternalInput").ap()
    ones_ap = nc.dram_tensor("onesin", [P, P], BF16, kind="ExternalInput").ap()
    # output: row gt*128+p, col (2*mc+ri)*512+tok  (r/i interleaved per mc)
    po_ap = nc.dram_tensor("po", [NT * P, 2 * DC * TBLK], BF16,
                           kind="ExternalOutput").ap()

    with tile.TileContext(nc) as tc, ExitStack() as ctx:
        wpool = ctx.enter_context(tc.tile_pool(name="w", bufs=1))
        xpool = ctx.enter_context(tc.tile_pool(name="x", bufs=8))
        qkpool = ctx.enter_context(tc.tile_pool(name="qk", bufs=2))
        vpool = ctx.enter_context(tc.tile_pool(name="v", bufs=2))
        opool = ctx.enter_context(tc.tile_pool(name="ost", bufs=2))
        upool = ctx.enter_context(tc.tile_pool(name="u", bufs=3))
        uaccpool = ctx.enter_context(tc.tile_pool(name="uacc", bufs=2))
        zpool = ctx.enter_context(tc.tile_pool(name="z", bufs=2))
        tmppool = ctx.enter_context(tc.tile_pool(name="tmp", bufs=2))
        popool = ctx.enter_context(tc.tile_pool(name="po", bufs=2))
        # PSUM: scores 2x2 banks + AV wide 1x2 banks + aux 2x1 bank = 8
        sps = ctx.enter_context(tc.tile_pool(name="sp", bufs=2, space="PSUM"))
        avps = ctx.enter_context(tc.tile_pool(name="av", bufs=1,
                                              space="PSUM"))
        auxps = ctx.enter_context(tc.tile_pool(name="ax", bufs=2,
                                               space="PSUM"))

        # Startup DMAs: weights on the scalar HWDGE queue in consumption
        # order (wvt, wq, wk, wo, ones), x loads on the sync queue in
        # consumption order — the two queues stream from HBM in parallel
        # and each arrival matches its consumer.
        wt = {}
        wot = {}
        wvt = {}
        xtiles = {}

        def load_w(dst, key, ap, pfx):
            dst[key] = wpool.tile(list(ap.shape), BF16, tag=f"{pfx}_{key}",
                                  name=f"{pfx}_{key}")
            nc.scalar.dma_start(dst[key][:], ap[:])

        def emit_xloads(b, t):
            # all x on the sync HWDGE queue in consumption order; weights
            # stream on the scalar queue in parallel.  (gpsimd is SWDGE,
            # ~55 GB/s — loads never go there.)
            for half in range(2):
                for part in ("r", "i"):
                    gt = 2 * b + half
                    xt = xpool.tile([P, DC * TBLK], BF16, tag="xt",
                                    name="xt")
                    nc.sync.dma_start(
                        xt[:], x_ap[t + part][gt * P:(gt + 1) * P, :])
                    xtiles[(b, t, part, half)] = xt

        # startup: interleave weight loads with batch-0 x loads so each
        # queue's arrival order matches proj(0)'s consumption order
        for part in ("r", "i"):
            load_w(wvt, part, wvt_ap[part], "wvt")
        emit_xloads(0, "v")
        for h in range(HPC):
            for suf in ("a", "b"):
                load_w(wt, f"q{suf}{h}", w_ap[f"q{suf}{h}"], "w")
        emit_xloads(0, "q")
        for h in range(HPC):
            for suf in ("a", "b"):
                load_w(wt, f"k{suf}{h}", w_ap[f"k{suf}{h}"], "w")
        emit_xloads(0, "k")
        for suf, ap in wo_ap.items():
            load_w(wot, suf, ap, "wo")
        ones = wpool.tile([P, P], BF16, tag="ones", name="ones")
        nc.scalar.dma_start(ones[:], ones_ap[:])

        def proj_gen(b, qcat, kcr, kci, va):
            """Projection of batch b as a stream of tensor quanta.

            Yields the emitted tensor-column count after each quantum.
            v first (V^T form), then q, then k; trailing vector ops of a
            psum group are emitted with its final quantum.
            """
            for half in range(2):
                xr = xtiles.pop((b, "v", "r", half))
                xi = xtiles.pop((b, "v", "i", half))
                for tb in range(4):
                    kc = half * 4 + tb
                    vps = auxps.tile([P, 2 * P], F32, tag="aux", name="vps")
                    for dcg in range(2):
                        for dc in range(dcg * 4, dcg * 4 + 4):
                            xs_ = slice(dc * TBLK + tb * P,
                                        dc * TBLK + (tb + 1) * P)
                            ws = slice(dc * 2 * P, (dc + 1) * 2 * P)
                            nc.tensor.matmul(
                                vps[:], xr[:, xs_], wvt["r"][:, ws],
                                start=(dc == 0), stop=False)
                            nc.tensor.matmul(
                                vps[:], xi[:, xs_], wvt["i"][:, ws],
                                start=False, stop=(dc == DC - 1))
                        if dcg == 1:
                            nc.vector.tensor_copy(
                                va[:, kc * 2 * P:(kc + 1) * 2 * P], vps[:])
                        yield 2048
            for t in ("q", "k"):
                for half in range(2):
                    xr = xtiles.pop((b, t, "r", half))
                    xi = xtiles.pop((b, t, "i", half))
                    hs = slice(half * TBLK, (half + 1) * TBLK)
                    for hh in range(2):
                        ps = auxps.tile([P, TBLK], F32, tag="aux",
                                        name="qkps")
                        wA = wt[f"{t}a{hh}"]
                        wB = wt[f"{t}b{hh}"]
                        for dcg in range(2):
                            for dc in range(dcg * 4, dcg * 4 + 4):
                                ws = slice(dc * P, (dc + 1) * P)
                                xs_ = slice(dc * TBLK, (dc + 1) * TBLK)
                                nc.tensor.matmul(
                                    ps[:], wA[:, ws], xr[:, xs_],
                                    start=(dc == 0), stop=False)
                                nc.tensor.matmul(
                                    ps[:], wB[:, ws], xi[:, xs_],
                                    start=False, stop=(dc == DC - 1))
                            if dcg == 1:
                                if t == "q":
                                    nc.vector.tensor_copy(qcat[hh][:, hs],
                                                          ps[:])
                                else:
                                    nc.vector.tensor_copy(kcr[hh][:, hs],
                                                          ps[:])
                                    nc.vector.tensor_scalar_mul(
                                        kci[hh][0:DH, hs], ps[DH:P, :], -1.0)
                                    nc.vector.tensor_copy(kci[hh][DH:P, hs],
                                                          ps[0:DH, :])
                            yield 4096

        def oproj_gen(b, o_stage, halves=(0, 1), pools=None, alt_copy=False):
            """O-projection of batch b as a stream of tensor quanta.

            po is stored in quarter chunks on the sync HWDGE queue so the
            final store's DMA tail is short and earlier chunks overlap
            compute.  `pools`: psum pools to rotate over and `alt_copy`
            alternates powide copies between vector and scalar (drain /
            thin-window mode: spreads the copies over both engines).
            """
            if pools is None:
                pools = ((auxps, "aux"),)
            pi_ = 0
            for half in halves:
                gt = 2 * b + half
                hs = slice(half * TBLK, (half + 1) * TBLK)
                powide = popool.tile([P, 2 * DC * TBLK], BF16, tag="pow",
                                     name="powide")
                for mc in range(DC):
                    ms = slice(mc * P, (mc + 1) * P)
                    for ri in range(2):
                        pool, ptag = pools[pi_ % len(pools)]
                        pi_ += 1
                        ps = pool.tile([P, TBLK], F32, tag=ptag,
                                       name="ops")
                        if ri == 0:
                            pairs = ((wot["r"], o_stage["r"]),
                                     (wot["in"], o_stage["i"]))
                        else:
                            pairs = ((wot["i"], o_stage["r"]),
                                     (wot["r"], o_stage["i"]))
                        nc.tensor.matmul(ps[:], pairs[0][0][:, ms],
                                         pairs[0][1][:, hs],
                                         start=True, stop=False)
                        nc.tensor.matmul(ps[:], pairs[1][0][:, ms],
                                         pairs[1][1][:, hs],
                                         start=False, stop=True)
                        c0 = (2 * mc + ri) * TBLK
                        if alt_copy and ri == 1:
                            nc.scalar.copy(powide[:, c0:c0 + TBLK], ps[:])
                        else:
                            nc.vector.tensor_copy(powide[:, c0:c0 + TBLK],
                                                  ps[:])
                        yield 1024
                    # store on the sync HWDGE queue — gpsimd SWDGE runs at
                    # ~55 GB/s and its final chunk would be the tail.
                    # Drain/thin mode (alt_copy) stores per-mc so the very
                    # last transfer is half as long.
                    if alt_copy or mc % 2 == 1:
                        lo = (2 * mc - (0 if alt_copy else 2)) * TBLK
                        cs = slice(lo, (2 * mc + 2) * TBLK)
                        nc.sync.dma_start(po_ap[gt * P:(gt + 1) * P, cs],
                                          powide[:, cs])

        def drain(gen):
            for _ in gen:
                pass

        def emit_window(b, qcat, kcr, kci, va, o_stage, filler,
                        mid_filler=None, xloader=None):
            """Attention units of batch b with filler interleaved.

            qb-outer unit order: both heads' q-half epilogues complete by
            mid-window, so `mid_filler` (last batch's own half-0 oproj)
            can be injected after unit 15.
            """
            units = [(h, qb, kc)
                     for qb in range(2) for h in range(HPC)
                     for kc in range(KC)]
            total_fill = (32768 if b >= 1 else 0) + \
                         (98304 if b + 1 < B else 0)
            per_unit = (total_fill + NUNITS - 1) // NUNITS
            swides = [None] * len(units)
            accs = {}
            budget = 0

            def emit_scores(n):
                h, qb, kc = units[n]
                qs = slice(qb * TBLK, (qb + 1) * TBLK)
                ks = slice(kc * P, (kc + 1) * P)
                sw = sps.tile([P, WBLK], F32, tag="sps", name="scorew")
                nc.tensor.matmul(sw[:, 0:TBLK], kcr[h][:, ks],
                                 qcat[h][:, qs], start=True, stop=True)
                nc.tensor.matmul(sw[:, TBLK:WBLK], kci[h][:, ks],
                                 qcat[h][:, qs], start=True, stop=True)
                swides[n] = sw

            def epilogue_z(h, qb, uacc, avw):
                # stage 1: Z = ones^T u_acc (one matmul pair per (h,qb))
                # and the reciprocals.  Runs one unit after the group's
                # last exp -> u_acc add, so the chain never head-of-line
                # blocks the tensor queue.
                zps_r = auxps.tile([P, TBLK], F32, tag="aux", name="zpr")
                nc.tensor.matmul(zps_r[:], ones[:], uacc[:, 0:TBLK],
                                 start=True, stop=True)
                zps_i = auxps.tile([P, TBLK], F32, tag="aux", name="zpi")
                nc.tensor.matmul(zps_i[:], ones[:], uacc[:, TBLK:WBLK],
                                 start=True, stop=True)
                zinv = zpool.tile([P, WBLK], F32, tag="zinv", name="zinv")
                nc.vector.reciprocal_approx_fast(zinv[:, 0:TBLK], zps_r[:])
                nc.vector.reciprocal_approx_fast(zinv[:, TBLK:WBLK],
                                                 zps_i[:])
                return zinv

            def epilogue_comb(h, qb, avw, zinv, comb=None):
                # stage 2 (a filler's worth later, right before the next
                # group's AV needs the avw buffer):
                # o_r = (v_r.T u_r)/Z_r - (v_i.T u_i)/Z_i etc.
                # ob (= avw cols TBLK:) is va^T u_i, halves swapped in
                # the combine; psum+sbuf DVE inputs are exempt from
                # the same-base-partition rule.  The final sub/add are
                # SBUF-only, so `comb` can route them to gpsimd to
                # relieve the vector queue (not for the last group — its
                # o_stage write gates the drain and gpsimd is slower).
                if comb is None:
                    comb = nc.vector
                qs = slice(qb * TBLK, (qb + 1) * TBLK)
                tmpa = tmppool.tile([P, TBLK], F32, tag="tmpa", name="tmpa")
                nc.vector.tensor_mul(tmpa[:], avw[:, 0:TBLK],
                                     zinv[:, 0:TBLK])
                tmpb = tmppool.tile([P, TBLK], F32, tag="tmpb", name="tmpb")
                nc.vector.tensor_mul(tmpb[0:DH, :], avw[DH:P, TBLK:WBLK],
                                     zinv[DH:P, TBLK:WBLK])
                nc.vector.tensor_mul(tmpb[DH:P, :], avw[0:DH, TBLK:WBLK],
                                     zinv[0:DH, TBLK:WBLK])
                dst = slice(DH * h, DH * (h + 1))
                comb.tensor_sub(o_stage["r"][dst, qs],
                                tmpa[0:DH, :], tmpb[0:DH, :])
                comb.tensor_add(o_stage["i"][dst, qs],
                                tmpa[DH:P, :], tmpb[DH:P, :])

            pending = None
            emit_scores(0)
            for n, (h, qb, kc) in enumerate(units):
                while xloader and xloader[0][0] <= n:
                    xloader.pop(0)[1]()
                if n + 1 < len(units):
                    emit_scores(n + 1)
                if n == NUNITS // 2 and mid_filler is not None:
                    filler.append(mid_filler)
                    per_unit += 1024
                first, last = kc == 0, kc == KC - 1
                if first:
                    uacc = uaccpool.tile([P, WBLK], BF16, tag="uacc",
                                         name="uacc")
                    avw = avps.tile([P, WBLK], F32, tag="av", name="avw")
                    accs[(h, qb)] = (uacc, avw)
                    u = uacc
                    nc.scalar.activation(uacc[:], swides[n][:], EXP)
                else:
                    uacc, avw = accs[(h, qb)]
                    u = upool.tile([P, WBLK], BF16, tag="u", name="u")
                    nc.scalar.activation(u[:], swides[n][:], EXP)
                    nc.vector.tensor_add(uacc[:], uacc[:], u[:])
                swides[n] = None
                # previous group's deferred Z/recip (stage 1)
                if pending is not None:
                    ph, pqb, puacc, pavw = pending
                    pzinv = epilogue_z(ph, pqb, puacc, pavw)
                # filler; kc==0 units get a bonus pop so the avw-reuse
                # boundary (previous group's deferred epilogue) is hidden.
                # 128 cols/unit are withheld here and spent after stage 2
                # below (budget-NEUTRAL relocation — overdrawing the
                # filler total leaves window-tail units scalar-paced).
                budget += per_unit - 128 + (2048 if first else 0)
                while budget > 0 and filler:
                    try:
                        budget -= next(filler[0])
                    except StopIteration:
                        filler.pop(0)
                # stage 2 after the filler: the avw psum buffer frees just
                # before this unit's AV (which reuses it when first); the
                # withheld budget lands here to cover the muls' latency
                if pending is not None:
                    epilogue_comb(ph, pqb, pavw, pzinv, comb=nc.gpsimd)
                    pending = None
                    budget += 1024
                    while budget > 0 and filler:
                        try:
                            budget -= next(filler[0])
                        except StopIteration:
                            filler.pop(0)
                # AV
                vsl = va[:, kc * 2 * P + h * P:kc * 2 * P + (h + 1) * P]
                nc.tensor.matmul(avw[:, 0:TBLK], vsl, u[:, 0:TBLK],
                                 start=first, stop=last)
                nc.tensor.matmul(avw[:, TBLK:WBLK], vsl, u[:, TBLK:WBLK],
                                 start=first, stop=last)
                if last:
                    pending = (h, qb, uacc, avw)
                    del accs[(h, qb)]
            pzinv = epilogue_z(*pending)
            epilogue_comb(pending[0], pending[1], pending[3], pzinv)
            # drain leftover filler
            for g in filler:
                drain(g)

        # ---- pipelined emission: one continuous tensor stream ----
        stage = {}

        def new_stage(b):
            qcat = [qkpool.tile([P, S], BF16, tag=f"qcat{h}", name=f"qcat{h}")
                    for h in range(HPC)]
            kcr = [qkpool.tile([P, S], BF16, tag=f"kcr{h}", name=f"kcr{h}")
                   for h in range(HPC)]
            kci = [qkpool.tile([P, S], BF16, tag=f"kci{h}", name=f"kci{h}")
                   for h in range(HPC)]
            # va: [128 tokens-in-chunk, kc*256 + h*128 + [v_r(64)|v_i(64)]]
            va = vpool.tile([P, 2 * S], BF16, tag="va", name="va")
            o_stage = {p: opool.tile([P, S], BF16, tag=f"ost{p}",
                                     name=f"ost{p}")
                       for p in ("r", "i")}
            stage[b] = (qcat, kcr, kci, va, o_stage)

        new_stage(0)
        drain(proj_gen(0, *stage[0][:4]))
        for b in range(B):
            xloader = None
            if b + 1 < B:
                # v loads issue before the window; q/k issue mid-window so
                # their descriptor-trigger instructions don't delay the
                # scalar queue's first exps
                emit_xloads(b + 1, "v")
                xloader = [(6, lambda bb=b + 1: emit_xloads(bb, "q")),
                           (14, lambda bb=b + 1: emit_xloads(bb, "k"))]
                new_stage(b + 1)
            filler = []
            if b >= 1:
                # last window has no proj filler and is vector/scalar
                # paced: spread its oproj copies over both engines
                filler.append(oproj_gen(b - 1, stage[b - 1][4],
                                        alt_copy=(b == B - 1)))
            if b + 1 < B:
                filler.append(proj_gen(b + 1, *stage[b + 1][:4]))
            # last window: inject this batch's own half-0 oproj once both
            # q-half-0 epilogues are in (qb-outer order, after unit 15)
            mid = (oproj_gen(b, stage[b][4], halves=(0,), alt_copy=True)
                   if b == B - 1 else None)
            emit_window(b, *stage[b], filler, mid_filler=mid,
                        xloader=xloader)
            if b >= 1:
                del stage[b - 1]
        drain(oproj_gen(B - 1, stage[B - 1][4], halves=(1,),
                        pools=((auxps, "aux"), (sps, "sps")),
                        alt_copy=True))

    nc.compile()
    return nc


def _w_sbuf_layout(w_t):
    """[D, 128] weight-transpose slice -> SBUF layout [128, dc*128+o]."""
    return np.ascontiguousarray(
        w_t.reshape(DC, P, P).transpose(1, 0, 2).reshape(P, D))


def _tile_x(xT, dtype):
    """[D, B*S] -> partition-major [NT*P, DC*TBLK] (row gt*P+p, col dc*TBLK+t)."""
    t = xT.reshape(DC, P, NT, TBLK).transpose(2, 1, 0, 3)
    return np.ascontiguousarray(t.reshape(NT * P, DC * TBLK)).astype(dtype)


def _prepare_in_maps(inputs):
    bf = ml_dtypes.bfloat16
    xs = {}
    for name, t in (("queries", "q"), ("keys", "k"), ("values", "v")):
        x = np.asarray(inputs[name], dtype=np.float32)  # [B,S,D,2]
        flat = x.reshape(B * S, D, 2)
        xs[t + "r"] = _tile_x(flat[:, :, 0].T, bf)
        xs[t + "i"] = _tile_x(flat[:, :, 1].T, bf)

    scale = np.float32(1.0 / np.sqrt(DH))
    in_maps = []
    for c in range(NCORES):
        rows = slice(P * c, P * (c + 1))
        m = {}
        for t in ("q", "k", "v"):
            for part in ("r", "i"):
                m[f"x{t}_{part}"] = xs[t + part]
        for t, wr_name, wi_name in (("q", "wq_r", "wq_i"),
                                    ("k", "wk_r", "wk_i")):
            s = scale if t == "q" else np.float32(1.0)
            wr = np.asarray(inputs[wr_name], dtype=np.float32)[rows] * s
            wi = np.asarray(inputs[wi_name], dtype=np.float32)[rows] * s
            for h in range(HPC):
                hr = slice(DH * h, DH * (h + 1))
                if t == "q":
                    wa = np.concatenate([wr[hr].T, wi[hr].T], axis=1)
                    wb = np.concatenate([-wi[hr].T, wr[hr].T], axis=1)
                else:
                    wa = np.concatenate([wr[hr].T, -wi[hr].T], axis=1)
                    wb = np.concatenate([-wi[hr].T, -wr[hr].T], axis=1)
                m[f"w{t}_a{h}"] = _w_sbuf_layout(wa).astype(bf)
                m[f"w{t}_b{h}"] = _w_sbuf_layout(wb).astype(bf)
        # V^T weights, moving operand: [1024 d, 2 heads * (v_r 64 | v_i 64)]
        # chunked to [128, dc*256 + c]
        wvr = np.asarray(inputs["wv_r"], dtype=np.float32)[rows]  # [128,1024]
        wvi = np.asarray(inputs["wv_i"], dtype=np.float32)[rows]
        br = np.concatenate(
            [np.concatenate([wvr[DH * h:DH * (h + 1)].T,
                             wvi[DH * h:DH * (h + 1)].T], axis=1)
             for h in range(HPC)], axis=1)  # [1024, 256]
        bi = np.concatenate(
            [np.concatenate([-wvi[DH * h:DH * (h + 1)].T,
                             wvr[DH * h:DH * (h + 1)].T], axis=1)
             for h in range(HPC)], axis=1)
        m["wvt_r"] = np.ascontiguousarray(
            br.reshape(DC, P, 2 * P).transpose(1, 0, 2).reshape(
                P, DC * 2 * P)).astype(bf)
        m["wvt_i"] = np.ascontiguousarray(
            bi.reshape(DC, P, 2 * P).transpose(1, 0, 2).reshape(
                P, DC * 2 * P)).astype(bf)
        wo_r = np.asarray(inputs["wo_r"], dtype=np.float32)[:, rows]  # [D,128]
        wo_i = np.asarray(inputs["wo_i"], dtype=np.float32)[:, rows]
        m["wo_r"] = np.ascontiguousarray(wo_r.T).astype(bf)  # [128 d, 1024 m]
        m["wo_i"] = np.ascontiguousarray(wo_i.T).astype(bf)
        m["wo_in"] = np.ascontiguousarray(-wo_i.T).astype(bf)
        m["onesin"] = np.ones((P, P), dtype=bf)
        in_maps.append(m)
    return in_maps


LAST_RESULT = None


def _run(inputs, trace=False):
    global LAST_RESULT
    from concourse.bass_utils import run_bass_kernel_spmd
    if "nc" not in _CACHE:
        _CACHE["nc"] = _build()
    nc = _CACHE["nc"]
    in_maps = _prepare_in_maps(inputs)
    if trace:
        os.environ.pop("BASS_NEVER_TRACE", None)
    else:
        os.environ["BASS_NEVER_TRACE"] = "1"
    res = run_bass_kernel_spmd(nc, in_maps, core_ids=list(range(NCORES)),
                               trace=trace)
    LAST_RESULT = res
    # po rows gt*P+p, cols (2*mc+ri)*TBLK+tok
    acc = np.zeros((NT * P, 2 * DC * TBLK), np.float32)
    for c in range(NCORES):
        acc += res.results[c]["po"].astype(np.float32)

    t = acc.reshape(NT, P, DC, 2, TBLK)
    out = np.empty((B, S, D, 2), np.float32)
    for ri in range(2):
        # value at [gt, p, mc, ri, tok] = out_part[d=mc*128+p, gt*512+tok]
        comp = t[:, :, :, ri, :].transpose(2, 1, 0, 3).reshape(D, B * S)
        out[..., ri] = comp.T.reshape(B, S, D)
    return out


def kernel(**inputs):
    return _run(inputs, trace=False)
